# revision 1
# baseline (speedup 1.0000x reference)
"""Distance-aware multihead attention on 8 Trainium2 NeuronCores.

Problem: B=4, S=1024, D=768, H=12, DK=64, NUM_EMB=10.
  q/k/v = linear projections of query/key/value
  idx[b,i,j] = clip(round(9 * |pos_i - pos_j| / MAXD), 0, 9)
  logits = (q.k^T + qe[b,h,i,idx[b,i,j]]) / 8   where qe = q @ emb_k^T
  out = softmax(logits) @ v

Key decompositions:
  - bias qe[...,idx] = qe[...,0] + sum_{e=1..9} (qe_e - qe_{e-1}) * (d2 >= T_e^2);
    the qe_0 term is constant along the softmax axis and cancels -> dropped.
  - step masks (d2 >= T_e^2) are shared across all 12 heads of a q-tile.
  - bias accumulated onto QK logits via 9 scalar_tensor_tensor ops per (head, q-tile).

Sharding: core c handles batch c//2, query-half c%2 (512 queries, all heads).
K/V/projections are computed per-core from full-S inputs (duplicated across the
2 cores sharing a batch); masks/logits/AV are not duplicated.

Layouts: Q^T/K^T [dim, token] f32r (from projections), V [token, dim] bf16.
P = exp((qk+bias)/8) bf16 in [q, k]; transposed to [k, q] 128-chunks via the
DMA-xbar transpose engine; AV accumulates over the 8 k-chunks on TensorE.
"""
import os
import numpy as np

import concourse.bass as bass
import concourse.tile as tile
from concourse import bacc, mybir
from concourse.bass_utils import run_bass_kernel_spmd

F32 = mybir.dt.float32
F32R = mybir.dt.float32r
BF16 = mybir.dt.bfloat16
ACT = mybir.ActivationFunctionType
ALU = mybir.AluOpType

B, S, D = 4, 1024, 768
H, DK = 12, 64
NUM_EMB = 10
MAX_DIST = 100000.0 * 2 ** 0.5
SQ = S // 2          # queries per core
NQT = SQ // 128      # q-tiles per core (4)
NKT = S // 128       # k token chunks (8)
NDT = D // 128       # dim tiles (6)
NCORES = 8

# squared thresholds: idx >= e  <=>  d2 >= ((e-0.5)*MAX_DIST/9)^2
THRESH2 = [float(((e - 0.5) * MAX_DIST / 9.0) ** 2) for e in range(1, NUM_EMB)]


def _load_T(nc, dst, src_dram, ncols):
    """src [rows, ncols*64] DRAM -> dst [128, ncols_grp, rows] = src^T, via
    64-partition xbar transpose chunks. dst is [128, n, rows] with
    dst[(64j)%128 + p64, j//2, r] = src[r, 64j + p64]."""
    for j in range(ncols // 64):
        nc.sync.dma_start_transpose(
            dst[(64 * j) % 128:(64 * j) % 128 + 64, j // 2, :],
            src_dram[:, 64 * j:64 * j + 64])


def build_nc(stage="full"):
    nc = bacc.Bacc("TRN2", target_bir_lowering=False, debug=False)

    # matmul-feeding inputs are float32r so the fp32r verifier accepts
    # DMA -> SBUF -> matmul (host values are plain fp32 bits).
    xq = nc.dram_tensor("xq", [SQ, D], F32R, kind="ExternalInput").ap()
    xk = nc.dram_tensor("xk", [S, D], F32R, kind="ExternalInput").ap()
    xv = nc.dram_tensor("xv", [S, D], F32R, kind="ExternalInput").ap()
    pos = nc.dram_tensor("pos", [S, 2], F32, kind="ExternalInput").ap()
    posq = nc.dram_tensor("posq", [SQ, 2], F32, kind="ExternalInput").ap()
    wq = nc.dram_tensor("wq", [D, D], F32R, kind="ExternalInput").ap()
    wk = nc.dram_tensor("wk", [D, D], F32R, kind="ExternalInput").ap()
    wv = nc.dram_tensor("wv", [D, D], F32R, kind="ExternalInput").ap()
    bq = nc.dram_tensor("bq", [D], F32, kind="ExternalInput").ap()
    bk = nc.dram_tensor("bk", [D], F32, kind="ExternalInput").ap()
    bv = nc.dram_tensor("bv", [D], F32, kind="ExternalInput").ap()
    emb = nc.dram_tensor("emb", [NUM_EMB, DK], F32R, kind="ExternalInput").ap()
    out = nc.dram_tensor("out", [SQ, D], F32, kind="ExternalOutput").ap()

    # debug stages: "proj" stops after projections, "masks" after d2/masks,
    # "logits" skips transpose+AV, "notrans" replaces the P transpose with a
    # plain DMA (wrong values, isolates the xbar), "full" is the real kernel.
    with tile.TileContext(nc) as tc:
        with tc.tile_pool(name="persist", bufs=1) as persist:
            # ---- setup: bias columns, position broadcasts ----
            bq_col = persist.tile([128, NDT], F32)
            bk_col = persist.tile([128, NDT], F32)
            nc.sync.dma_start(out=bq_col[:], in_=bass.AP(tensor=bq.tensor, offset=0, ap=[[1, 128], [128, NDT]]))
            nc.sync.dma_start(out=bk_col[:], in_=bass.AP(tensor=bk.tensor, offset=0, ap=[[1, 128], [128, NDT]]))
            bv_b = persist.tile([128, D], F32)
            nc.sync.dma_start(out=bv_b[:], in_=bass.AP(tensor=bv.tensor, offset=0, ap=[[0, 128], [1, D]]))
            xk_b = persist.tile([128, S], F32)
            yk_b = persist.tile([128, S], F32)
            nc.sync.dma_start(out=xk_b[:], in_=bass.AP(tensor=pos.tensor, offset=0, ap=[[0, 128], [2, S]]))
            nc.sync.dma_start(out=yk_b[:], in_=bass.AP(tensor=pos.tensor, offset=1, ap=[[0, 128], [2, S]]))
            # query positions as per-partition scalars [128, NQT]
            xq_col = persist.tile([128, NQT], F32)
            yq_col = persist.tile([128, NQT], F32)
            nc.sync.dma_start(out=xq_col[:], in_=bass.AP(tensor=posq.tensor, offset=0, ap=[[2, 128], [256, NQT]]))
            nc.sync.dma_start(out=yq_col[:], in_=bass.AP(tensor=posq.tensor, offset=1, ap=[[2, 128], [256, NQT]]))
            # emb^T on both 64-partition halves
            embT = persist.tile([128, NUM_EMB], F32R)
            nc.sync.dma_start_transpose(embT[0:64, :], emb[:, :])
            nc.sync.dma_start_transpose(embT[64:128, :], emb[:, :])
            embT_blk = persist.tile([128, 2 * NUM_EMB], F32R)
            nc.vector.memset(embT_blk[:].bitcast(F32), 0.0)
            nc.sync.dma_start_transpose(embT_blk[0:64, 0:NUM_EMB], emb[:, :])
            nc.sync.dma_start_transpose(embT_blk[64:128, NUM_EMB:2 * NUM_EMB], emb[:, :])

            ident = persist.tile([128, 128], BF16)
            from concourse.masks import make_identity
            make_identity(nc, ident[:])
            v_sb = persist.tile([128, NKT, D], BF16)   # V[token, dim], token-chunked
            kT = persist.tile([128, NDT, S], F32R)     # K^T[dim, token]
            qT = persist.tile([128, NDT, SQ], F32R)    # Q^T[dim, token]

            # ---- projections (phased so X^T/W^T buffers are freed early) ----
            with tc.tile_pool(name="vproj", bufs=1) as vp, \
                 tc.tile_pool(name="vps", bufs=2, space="PSUM") as vps:
                wvT = vp.tile([128, NDT, D], F32R)
                xvT = vp.tile([128, NDT, S], F32R)
                _load_T(nc, wvT, wv, D)
                _load_T(nc, xvT, xv, D)
                for m in range(NKT):
                    for hf in range(2):
                        ps = vps.tile([128, 384], F32, tag="pj")
                        for t in range(NDT):
                            nc.tensor.matmul(ps[:], xvT[:, t, 128 * m:128 * m + 128],
                                             wvT[:, t, 384 * hf:384 * hf + 384],
                                             start=(t == 0), stop=(t == NDT - 1))
                        nc.scalar.copy(v_sb[:, m, 384 * hf:384 * hf + 384], ps[:])

            with tc.tile_pool(name="kproj", bufs=1) as kp, \
                 tc.tile_pool(name="kps", bufs=2, space="PSUM") as kps:
                wkT = kp.tile([128, NDT, D], F32R)
                xkT = kp.tile([128, NDT, S], F32R)
                _load_T(nc, wkT, wk, D)
                _load_T(nc, xkT, xk, D)
                for m in range(NDT):
                    for hf in range(2):
                        ps = kps.tile([128, 512], F32, tag="pj")
                        for t in range(NDT):
                            nc.tensor.matmul(ps[:], wkT[:, t, 128 * m:128 * m + 128],
                                             xkT[:, t, 512 * hf:512 * hf + 512],
                                             start=(t == 0), stop=(t == NDT - 1))
                        nc.scalar.activation(kT[:, m, 512 * hf:512 * hf + 512], ps[:],
                                             ACT.Identity, bias=bk_col[:, m:m + 1])

            with tc.tile_pool(name="qproj", bufs=1) as qp, \
                 tc.tile_pool(name="qps", bufs=2, space="PSUM") as qps:
                wqT = qp.tile([128, NDT, D], F32R)
                xqT = qp.tile([128, NDT, SQ], F32R)
                _load_T(nc, wqT, wq, D)
                _load_T(nc, xqT, xq, D)
                for m in range(NDT):
                    ps = qps.tile([128, 512], F32, tag="pj")
                    for t in range(NDT):
                        nc.tensor.matmul(ps[:], wqT[:, t, 128 * m:128 * m + 128],
                                         xqT[:, t, :],
                                         start=(t == 0), stop=(t == NDT - 1))
                    nc.scalar.activation(qT[:, m, :], ps[:], ACT.Identity,
                                         bias=bq_col[:, m:m + 1])

            if stage == "proj":
                # dump some projection results and stop
                with tc.tile_pool(name="dump", bufs=1) as dp:
                    t = dp.tile([128, 512], F32)
                    nc.scalar.copy(t[:], qT[:, 0, :].bitcast(F32))
                    nc.sync.dma_start(out=out[0:128, 0:512], in_=t[:])
                    t2 = dp.tile([128, 512], F32)
                    nc.scalar.copy(t2[:], kT[:, 0, 0:512].bitcast(F32))
                    nc.sync.dma_start(out=out[128:256, 0:512], in_=t2[:])
                    t3 = dp.tile([128, 512], F32)
                    nc.vector.tensor_copy(t3[:], v_sb[:, 0, 0:512])
                    nc.sync.dma_start(out=out[256:384, 0:512], in_=t3[:])

            # ---- attention ----
            if os.environ.get("BARRIER"):
                tc.strict_bb_all_engine_barrier()
            if not os.environ.get("NOWARMXP"):
                # dummy 2-byte xbar transpose: the first 2B transpose after the
                # 4B setup transposes produces garbage (xbar mode transition);
                # this one absorbs it.
                scrap = persist.tile([128, 128], BF16)
                scrapT = persist.tile([128, 128], BF16)
                nc.vector.memset(scrap[:], 0.0)
                nc.sync.dma_start_transpose(scrapT[:], scrap[:])
            if stage != "proj":
              with tc.tile_pool(name="att", bufs=2) as att, \
                 tc.tile_pool(name="accp", bufs=2) as accp, \
                 tc.tile_pool(name="qe_ps", bufs=1, space="PSUM") as qe_ps, \
                 tc.tile_pool(name="qk_ps", bufs=2, space="PSUM") as qk_ps, \
                 tc.tile_pool(name="pt_ps", bufs=1, space="PSUM") as pt_ps, \
                 tc.tile_pool(name="av_ps", bufs=2, space="PSUM") as av_ps:
                for qt in range(1 if os.environ.get("NQT1") else (NQT if (stage not in ("masks", "logits", "d2") or os.environ.get("FULLLOOPS")) else 1)):
                    if os.environ.get("QTBARRIER"):
                        tc.strict_bb_all_engine_barrier()
                    if os.environ.get("NOMASKS"):
                        masks = att.tile([128, NUM_EMB - 1, S], BF16, tag="masks")
                        dqe = att.tile([128, H, NUM_EMB - 1], F32, tag="dqe")
                        if os.environ.get("DOD2"):
                            dx = att.tile([128, S], F32, tag="dx")
                            dy = att.tile([128, S], F32, tag="dy")
                            nc.vector.tensor_scalar(out=dx[:], in0=xk_b[:], scalar1=xq_col[:, qt:qt + 1],
                                                    scalar2=None, op0=ALU.subtract)
                            nc.vector.tensor_scalar(out=dy[:], in0=yk_b[:], scalar1=yq_col[:, qt:qt + 1],
                                                    scalar2=None, op0=ALU.subtract)
                            dx2 = att.tile([128, S], F32, tag="dx2")
                            dy2 = att.tile([128, S], F32, tag="dy2")
                            nc.scalar.square(dx2[:], dx[:])
                            nc.scalar.square(dy2[:], dy[:])
                            d2 = att.tile([128, S], F32, tag="d2")
                            nc.vector.tensor_add(d2[:], dx2[:], dy2[:])
                            if os.environ.get("DOMASKS"):
                                for e in range(NUM_EMB - 1):
                                    nc.vector.tensor_scalar(out=masks[:, e, :], in0=d2[:],
                                                            scalar1=THRESH2[e], scalar2=None,
                                                            op0=ALU.is_ge)
                        if os.environ.get("SECTBARRIER"):
                            tc.strict_bb_all_engine_barrier()
                        if os.environ.get("DOQE"):
                            qe_psum = qe_ps.tile([128, H * NUM_EMB], F32, tag="qe")
                            if os.environ.get("QEBLK"):
                                for m in range(NDT):
                                    nc.tensor.matmul(qe_psum[:, 20 * m:20 * m + 20],
                                                     qT[:, m, 128 * qt:128 * qt + 128],
                                                     embT_blk[:],
                                                     start=True, stop=True)
                            else:
                                for h in range(H):
                                    off = (64 * h) % 128
                                    nc.tensor.matmul(qe_psum[:, 10 * h:10 * h + 10],
                                                     qT[off:off + 64, h // 2, 128 * qt:128 * qt + 128],
                                                     embT[off:off + 64, :],
                                                     start=True, stop=True)
                            qe_sb = att.tile([128, H, NUM_EMB], F32, tag="qe_sb")
                            nc.scalar.copy(qe_sb[:], qe_psum[:].rearrange("p (h e) -> p h e", e=NUM_EMB))
                            nc.vector.tensor_tensor(out=dqe[:], in0=qe_sb[:, :, 1:],
                                                    in1=qe_sb[:, :, :-1], op=ALU.subtract)
                        if os.environ.get("SECTBARRIER"):
                            tc.strict_bb_all_engine_barrier()
                        for h in range(H):
                            off = 0 if os.environ.get("OFF0") else (64 * h) % 128
                            qk = qk_ps.tile([128, S], F32, tag="qk")
                            for hf in range(2):
                                nc.tensor.matmul(qk[:, 512 * hf:512 * hf + 512],
                                                 qT[off:off + 64, h // 2, 128 * qt:128 * qt + 128],
                                                 kT[off:off + 64, h // 2, 512 * hf:512 * hf + 512],
                                                 start=True, stop=True)
                            o3 = att.tile([128, DK], F32, tag="o")
                            nc.scalar.copy(o3[:], qk[:, 0:DK])
                            nc.sync.dma_start(out=out[128 * qt:128 * qt + 128, 64 * h:64 * h + 64],
                                              in_=o3[:])
                        continue
                    # --- d2 for this q-tile: [128, S] fp32 ---
                    dx = att.tile([128, S], F32, tag="dx")
                    dy = att.tile([128, S], F32, tag="dy")
                    nc.vector.tensor_scalar(out=dx[:], in0=xk_b[:], scalar1=xq_col[:, qt:qt + 1],
                                            scalar2=None, op0=ALU.subtract)
                    nc.vector.tensor_scalar(out=dy[:], in0=yk_b[:], scalar1=yq_col[:, qt:qt + 1],
                                            scalar2=None, op0=ALU.subtract)
                    dx2 = att.tile([128, S], F32, tag="dx2")
                    dy2 = att.tile([128, S], F32, tag="dy2")
                    nc.scalar.square(dx2[:], dx[:])
                    nc.scalar.square(dy2[:], dy[:])
                    d2 = att.tile([128, S], F32, tag="d2")
                    nc.vector.tensor_add(d2[:], dx2[:], dy2[:])

                    if stage == "qeonly":
                        qe_psum = qe_ps.tile([128, H * NUM_EMB], F32, tag="qe")
                        for h in range(H):
                            off = (64 * h) % 128
                            nc.tensor.matmul(qe_psum[:, 10 * h:10 * h + 10],
                                             qT[off:off + 64, h // 2, 128 * qt:128 * qt + 128],
                                             embT[off:off + 64, :],
                                             start=True, stop=True)
                        qe_sb = att.tile([128, H, NUM_EMB], F32, tag="qe_sb")
                        nc.scalar.copy(qe_sb[:], qe_psum[:].rearrange("p (h e) -> p h e", e=NUM_EMB))
                        dqe = att.tile([128, H, NUM_EMB - 1], F32, tag="dqe")
                        nc.vector.tensor_tensor(out=dqe[:], in0=qe_sb[:, :, 1:],
                                                in1=qe_sb[:, :, :-1], op=ALU.subtract)
                        o4 = att.tile([128, DK], F32, tag="o")
                        nc.vector.tensor_copy(o4[:, 0:63], dqe[:, 0:7, 0:9].rearrange("p a b -> p (a b)"))
                        nc.vector.tensor_copy(o4[:, 63:64], dqe[:, 7, 0:1])
                        nc.sync.dma_start(out=out[128 * qt:128 * qt + 128, 0:DK], in_=o4[:])
                        continue

                    if stage == "d2":
                        nc.sync.dma_start(out=out[128:256, 0:D], in_=d2[:, 0:D])
                        continue

                    # --- step masks [128, 9, S] bf16 ---
                    nmask = int(os.environ.get("NMASKS", str(NUM_EMB - 1)))
                    mdt = F32 if os.environ.get("MASKF32") else BF16
                    masks = att.tile([128, NUM_EMB - 1, S], mdt, tag="masks")
                    for e in range(nmask):
                        if os.environ.get("MASKCOPY"):
                            nc.vector.tensor_copy(masks[:, e, :], d2[:])
                        elif os.environ.get("MASKIMM1"):
                            nc.vector.tensor_scalar(out=masks[:, e, :], in0=d2[:],
                                                    scalar1=1.0, scalar2=None,
                                                    op0=ALU.is_ge)
                        else:
                            nc.vector.tensor_scalar(out=masks[:, e, :], in0=d2[:],
                                                    scalar1=THRESH2[e], scalar2=None,
                                                    op0=ALU.is_ge)

                    # --- qe -> dqe for this q-tile (block-diagonal: 2 heads per matmul;
                    # 64-partition sliver matmuls into one bank proved flaky on HW) ---
                    qe_psum = qe_ps.tile([128, H * NUM_EMB], F32, tag="qe")
                    for m in range(NDT):
                        nc.tensor.matmul(qe_psum[:, 20 * m:20 * m + 20],
                                         qT[:, m, 128 * qt:128 * qt + 128],
                                         embT_blk[:],
                                         start=True, stop=True)
                    qe_sb = att.tile([128, H, NUM_EMB], F32, tag="qe_sb")
                    nc.scalar.copy(qe_sb[:], qe_psum[:].rearrange("p (h e) -> p h e", e=NUM_EMB))
                    dqe = att.tile([128, H, NUM_EMB - 1], F32, tag="dqe")
                    nc.vector.tensor_tensor(out=dqe[:], in0=qe_sb[:, :, 1:],
                                            in1=qe_sb[:, :, :-1], op=ALU.subtract)

                    if stage == "masks":
                        if not os.environ.get("NODUMP"):
                            md = att.tile([128, S], F32, tag="md")
                            nc.vector.tensor_copy(md[:], masks[:, 0, :])
                            nc.sync.dma_start(out=out[0:128, 0:D], in_=md[:, 0:D])
                        nc.sync.dma_start(out=out[128:256, 0:D], in_=d2[:, 0:D])
                        continue

                    for h in range(H if (stage != "logits" or os.environ.get("FULLLOOPS")) else 1):
                        off = 0 if os.environ.get("OFF0") else (64 * h) % 128
                        # --- logits = q.k^T ---
                        qk = qk_ps.tile([128, S], F32, tag="qk")
                        for hf in range(2):
                            nc.tensor.matmul(qk[:, 512 * hf:512 * hf + 512],
                                             qT[off:off + 64, h // 2, 128 * qt:128 * qt + 128],
                                             kT[off:off + 64, h // 2, 512 * hf:512 * hf + 512],
                                             start=True, stop=True)
                        # --- + bias: 9 chained masked MACs ---
                        src = qk
                        if stage == "qkonly":
                            o3 = att.tile([128, DK], F32, tag="o")
                            nc.scalar.copy(o3[:], qk[:, 0:DK])
                            nc.sync.dma_start(out=out[128 * qt:128 * qt + 128, 64 * h:64 * h + 64],
                                              in_=o3[:])
                            continue
        
                        nstt = 0 if stage == "qkexp" else (NUM_EMB - 1)
                        for e in range(nstt):
                            acc = accp.tile([128, S], F32, tag="acc")
                            nc.vector.scalar_tensor_tensor(
                                out=acc[:], in0=masks[:, e, :], scalar=dqe[:, h, e:e + 1],
                                in1=src[:], op0=ALU.mult, op1=ALU.add)
                            src = acc
                        if stage == "sttonly":
                            o3 = att.tile([128, DK], F32, tag="o")
                            nc.vector.tensor_copy(o3[:], src[:, 0:DK])
                            nc.sync.dma_start(out=out[128 * qt:128 * qt + 128, 64 * h:64 * h + 64],
                                              in_=o3[:])
                            continue
                        # --- P = exp(logits/8), row-sum, transpose ---
                        p_sb = att.tile([128, S], BF16, tag="p")
                        den = att.tile([128, 1], F32, tag="den")
                        nc.scalar.activation(p_sb[:], src[:], ACT.Exp, scale=0.125,
                                             accum_out=den[:])
                        if stage in ("logits", "qkexp"):
                            pf = att.tile([128, S], F32, tag="pf")
                            nc.vector.tensor_copy(pf[:], p_sb[:])
                            nc.sync.dma_start(out=out[0:128, 0:D], in_=pf[:, 0:D])
                            continue
                        if os.environ.get("PSTAGE"):
                            p2 = att.tile([128, S], BF16, tag="p2")
                            nc.vector.tensor_copy(p2[:], p_sb[:])
                            p_sb = p2
                        pT = att.tile([128, NKT, 128], BF16, tag="pT")
                        if stage in ("notrans", "nopt", "av"):
                            nc.sync.dma_start(out=pT[:], in_=p_sb[:].rearrange("p (c j) -> p c j", j=128))
                        elif os.environ.get("XBARTRANS"):
                            # xbar transpose is only correct up to 512-wide inputs;
                            # first-op-in-kernel also glitches (see PE path below)
                            nc.sync.dma_start_transpose(pT[:, 0:NKT // 2, :], p_sb[:, 0:S // 2])
                            nc.sync.dma_start_transpose(pT[:, NKT // 2:NKT, :], p_sb[:, S // 2:S])
                        else:
                            ptp = pt_ps.tile([128, NKT, 128], BF16, tag="ptp")
                            for c in range(NKT):
                                nc.tensor.transpose(ptp[:, c, :], p_sb[:, 128 * c:128 * c + 128], ident[:])
                            nc.scalar.copy(pT[:], ptp[:])
                        # --- out_h = (P^T . V_h) / den + bv_h ---
                        if stage == "nopt":
                            # skip everything after exp except a pT dump
                            o2 = att.tile([128, DK], F32, tag="o")
                            nc.vector.tensor_copy(o2[:], pT[:, 0, 0:DK])
                            nc.sync.dma_start(out=out[128 * qt:128 * qt + 128, 64 * h:64 * h + 64],
                                              in_=o2[:])
                            continue
                        if os.environ.get("PTCOPY"):
                            pT2 = att.tile([128, NKT, 128], BF16, tag="pT2")
                            nc.vector.tensor_copy(pT2[:], pT[:])
                            pT = pT2
                        av = av_ps.tile([128, DK], F32, tag="av")
                        for c in range(NKT):
                            nc.tensor.matmul(av[:], pT[:, c, :], v_sb[:, c, 64 * h:64 * h + 64],
                                             start=(c == 0), stop=(c == NKT - 1))
                        if stage == "av":
                            o2 = att.tile([128, DK], F32, tag="o")
                            nc.scalar.copy(o2[:], av[:])
                            nc.sync.dma_start(out=out[128 * qt:128 * qt + 128, 64 * h:64 * h + 64],
                                              in_=o2[:])
                            continue
                        recip = att.tile([128, 1], F32, tag="recip")
                        nc.vector.reciprocal(recip[:], den[:])
                        o_sb = att.tile([128, DK], F32, tag="o")
                        nc.vector.scalar_tensor_tensor(
                            out=o_sb[:], in0=av[:], scalar=recip[:],
                            in1=bv_b[:, 64 * h:64 * h + 64], op0=ALU.mult, op1=ALU.add)
                        nc.sync.dma_start(out=out[128 * qt:128 * qt + 128, 64 * h:64 * h + 64],
                                          in_=o_sb[:])
    nc.compile()
    return nc


_NC_CACHE = {}


def _get_nc():
    if "nc" not in _NC_CACHE:
        _NC_CACHE["nc"] = build_nc()
    return _NC_CACHE["nc"]


def kernel(query, key, value, tile_positions, Wq, bq, Wk, bk, Wv, bv, emb_k):
    query = np.ascontiguousarray(np.asarray(query, dtype=np.float32))
    key = np.ascontiguousarray(np.asarray(key, dtype=np.float32))
    value = np.ascontiguousarray(np.asarray(value, dtype=np.float32))
    tile_positions = np.ascontiguousarray(np.asarray(tile_positions, dtype=np.float32))
    Wq = np.ascontiguousarray(np.asarray(Wq, dtype=np.float32))
    Wk = np.ascontiguousarray(np.asarray(Wk, dtype=np.float32))
    Wv = np.ascontiguousarray(np.asarray(Wv, dtype=np.float32))
    bq = np.ascontiguousarray(np.asarray(bq, dtype=np.float32))
    bk = np.ascontiguousarray(np.asarray(bk, dtype=np.float32))
    bv = np.ascontiguousarray(np.asarray(bv, dtype=np.float32))
    emb_k = np.ascontiguousarray(np.asarray(emb_k, dtype=np.float32))

    nc = _get_nc()
    in_maps = []
    for c in range(NCORES):
        b, qh = c // 2, c % 2
        in_maps.append({
            "xq": np.ascontiguousarray(query[b, qh * SQ:(qh + 1) * SQ]),
            "xk": key[b], "xv": value[b],
            "pos": tile_positions[b],
            "posq": np.ascontiguousarray(tile_positions[b, qh * SQ:(qh + 1) * SQ]),
            "wq": Wq, "wk": Wk, "wv": Wv,
            "bq": bq, "bk": bk, "bv": bv,
            "emb": emb_k,
        })
    res = run_bass_kernel_spmd(nc, in_maps, core_ids=list(range(NCORES)))
    out = np.empty((B, S, D), np.float32)
    for c in range(NCORES):
        b, qh = c // 2, c % 2
        out[b, qh * SQ:(qh + 1) * SQ] = res.results[c]["out"]
    return out



# revision 46
# speedup vs baseline: 11256.9232x; 11256.9232x over previous
"""Distance-aware multihead attention on 8 Trainium2 NeuronCores.

Problem: B=4, S=1024, D=768, H=12, DK=64, NUM_EMB=10.
  q/k/v = linear projections of query/key/value
  idx[b,i,j] = clip(round(9 * |pos_i - pos_j| / MAXD), 0, 9)
  logits = (q.k^T + qe[b,h,i,idx[b,i,j]]) / 8   where qe = q @ emb_k^T
  out = softmax(logits) @ v

Key decompositions:
  - bias qe[...,idx] = qe[...,0] + sum_{e=1..9} (qe_e - qe_{e-1}) * (d2 >= T_e^2);
    the qe_0 term is constant along the softmax axis and cancels -> dropped.
  - step masks (d2 >= T_e^2) are shared across all 12 heads of a q-tile.
  - bias accumulated onto QK logits via 9 scalar_tensor_tensor ops per (head,
    q-tile); the 12 head-chains are split across the DVE and Pool engines.

Sharding: core c handles batch c//2, query-half c%2 (512 queries, all heads).
K/V/projections are computed per-core from full-S inputs (duplicated across the
2 cores sharing a batch); masks/logits/AV are not duplicated.

Layouts: all matmul operands are loaded pre-transposed from DRAM (the host
transposes W/X/emb in kernel(), so every DMA is a wide contiguous copy — the
4-byte-granular xbar/broadcast DMA patterns this replaced were ~30x slower).
Q^T/K^T [dim, token] f32r, V [token, dim] bf16.
P = exp((qk+bias)/8) bf16 in [q, k]; transposed to [k, q] 128-chunks on the PE;
AV accumulates over the 8 k-chunks on TensorE.
"""
import os
import numpy as np

import concourse.bass as bass
import concourse.tile as tile
from concourse import bacc, mybir
from concourse.bass_utils import run_bass_kernel_spmd

F32 = mybir.dt.float32
F32R = mybir.dt.float32r
BF16 = mybir.dt.bfloat16
ACT = mybir.ActivationFunctionType
ALU = mybir.AluOpType

B, S, D = 4, 1024, 768
H, DK = 12, 64
NUM_EMB = 10
MAX_DIST = 100000.0 * 2 ** 0.5
SQ = S // 2          # queries per core
NQT = SQ // 128      # q-tiles per core (4)
NKT = S // 128       # k token chunks (8)
NDT = D // 128       # dim tiles (6)
NCORES = 8

# squared thresholds: idx >= e  <=>  d2 >= ((e-0.5)*MAX_DIST/9)^2
THRESH2 = [float(((e - 0.5) * MAX_DIST / 9.0) ** 2) for e in range(1, NUM_EMB)]

# bias-accumulation engine per head: most heads accumulate the step masks
# onto the qk PSUM via diag(dqe)-stationary matmuls on the PE; the heads
# listed here instead run f32 tensor-tensor add chains on DVE / Pool to
# balance engine load.
DVE_CHAIN_HEADS = frozenset(
    int(h) for h in os.environ.get("DVE_CHAIN_HEADS", "").split(",") if h != "")
POOL_CHAIN_HEADS = frozenset(
    int(h) for h in os.environ.get("POOL_CHAIN_HEADS", "3,7").split(",") if h != "")
# d2 stays f32: bf16 d2 misbuckets enough pairs to breach tolerance on HW
D2_DT = mybir.dt.float32


def _load_chunked(nc, dst, src_dram, rows, cols):
    """src [rows, cols] DRAM -> dst [128, rows//128, cols] SBUF with
    dst[p, g, c] = src[128*g + p, c]. One wide contiguous DMA."""
    g = rows // 128
    nc.sync.dma_start(
        out=dst[:],
        in_=bass.AP(tensor=src_dram.tensor, offset=0,
                    ap=[[cols, 128], [128 * cols, g], [1, cols]]))


def build_nc(stage="full"):
    nc = bacc.Bacc("TRN2", target_bir_lowering=False, debug=False)

    # matmul-feeding inputs are float32r so the fp32r verifier accepts
    # DMA -> SBUF -> matmul (host values are plain fp32 bits).
    xqT = nc.dram_tensor("xqT", [D, SQ], BF16, kind="ExternalInput").ap()
    xkT = nc.dram_tensor("xkT", [D, S], BF16, kind="ExternalInput").ap()
    xvT = nc.dram_tensor("xvT", [D, S], BF16, kind="ExternalInput").ap()
    pos = nc.dram_tensor("pos", [S, 2], F32, kind="ExternalInput").ap()
    posq = nc.dram_tensor("posq", [SQ, 2], F32, kind="ExternalInput").ap()
    wqT = nc.dram_tensor("wqT", [D, D], BF16, kind="ExternalInput").ap()
    wkT = nc.dram_tensor("wkT", [D, D], BF16, kind="ExternalInput").ap()
    wvT = nc.dram_tensor("wvT", [D, D], BF16, kind="ExternalInput").ap()
    bq = nc.dram_tensor("bq", [D], F32, kind="ExternalInput").ap()
    bk = nc.dram_tensor("bk", [D], F32, kind="ExternalInput").ap()
    bv = nc.dram_tensor("bv", [D], F32, kind="ExternalInput").ap()
    embT_h = nc.dram_tensor("embT_h", [DK, NUM_EMB], BF16, kind="ExternalInput").ap()
    out = nc.dram_tensor("out", [SQ, D], F32, kind="ExternalOutput").ap()

    with tile.TileContext(nc) as tc:
        with tc.tile_pool(name="persist", bufs=1) as persist:
            # ---- setup: bias columns, position broadcasts ----
            bq_col = persist.tile([128, NDT], F32)
            bk_col = persist.tile([128, NDT], F32)
            nc.sync.dma_start(out=bq_col[:], in_=bass.AP(tensor=bq.tensor, offset=0, ap=[[1, 128], [128, NDT]]))
            nc.sync.dma_start(out=bk_col[:], in_=bass.AP(tensor=bk.tensor, offset=0, ap=[[1, 128], [128, NDT]]))
            bv_b = persist.tile([128, D], F32)
            nc.sync.dma_start(out=bv_b[:], in_=bass.AP(tensor=bv.tensor, offset=0, ap=[[0, 128], [1, D]]))
            # key positions: load rows on partition 0, broadcast on Pool
            xrow = persist.tile([1, S], F32)
            yrow = persist.tile([1, S], F32)
            nc.sync.dma_start(out=xrow[:], in_=bass.AP(tensor=pos.tensor, offset=0, ap=[[2048, 1], [2, S]]))
            nc.sync.dma_start(out=yrow[:], in_=bass.AP(tensor=pos.tensor, offset=1, ap=[[2048, 1], [2, S]]))
            xk_b = persist.tile([128, S], F32)
            yk_b = persist.tile([128, S], F32)
            nc.gpsimd.partition_broadcast(xk_b[:], xrow[:])
            nc.gpsimd.partition_broadcast(yk_b[:], yrow[:])
            # query positions as per-partition scalars [128, NQT]
            xq_col = persist.tile([128, NQT], F32)
            yq_col = persist.tile([128, NQT], F32)
            nc.sync.dma_start(out=xq_col[:], in_=bass.AP(tensor=posq.tensor, offset=0, ap=[[2, 128], [256, NQT]]))
            nc.sync.dma_start(out=yq_col[:], in_=bass.AP(tensor=posq.tensor, offset=1, ap=[[2, 128], [256, NQT]]))
            # emb^T block-diagonal [128, 20]: [0:64,0:10] and [64:128,10:20]
            embT_blk = persist.tile([128, 2 * NUM_EMB], BF16)
            nc.vector.memset(embT_blk[:], 0.0)
            nc.sync.dma_start(out=embT_blk[0:64, 0:NUM_EMB],
                              in_=bass.AP(tensor=embT_h.tensor, offset=0, ap=[[NUM_EMB, 64], [1, NUM_EMB]]))
            nc.sync.dma_start(out=embT_blk[64:128, NUM_EMB:2 * NUM_EMB],
                              in_=bass.AP(tensor=embT_h.tensor, offset=0, ap=[[NUM_EMB, 64], [1, NUM_EMB]]))

            ident = persist.tile([128, 128], BF16)
            from concourse.masks import make_identity
            make_identity(nc, ident[:])
            # dummy 2-byte xbar transpose: the first xbar op in a kernel can
            # glitch (mode transition); this one absorbs it.
            scrap = persist.tile([128, 128], BF16)
            scrapT = persist.tile([128, 128], BF16)
            nc.vector.memset(scrap[:], 0.0)
            nc.sync.dma_start_transpose(scrapT[:], scrap[:])
            v_sb = persist.tile([128, NKT, D], BF16)   # V[token, dim], token-chunked
            kT = persist.tile([128, NDT, S], BF16)     # K^T[dim, token]
            qT = persist.tile([128, NDT, SQ], BF16)    # Q^T[dim, token]

            # ---- d2 prebuild ----
            # a small early pool so all four q-tiles' squared-distance rows
            # build on the (otherwise idle) DVE during the projection phase
            d2p_ctx = tc.tile_pool(name="d2p", bufs=1)
            d2p = d2p_ctx.__enter__()
            mb_ctx = tc.tile_pool(name="maskbuild", bufs=1)
            mb = mb_ctx.__enter__()

            def emit_d2(qt):
                dx = mb.tile([128, S], F32, tag="dx")
                dy = mb.tile([128, S], F32, tag="dy")
                nc.vector.tensor_scalar(out=dx[:], in0=xk_b[:], scalar1=xq_col[:, qt:qt + 1],
                                        scalar2=None, op0=ALU.subtract)
                nc.vector.tensor_scalar(out=dy[:], in0=yk_b[:], scalar1=yq_col[:, qt:qt + 1],
                                        scalar2=None, op0=ALU.subtract)
                nc.scalar.square(dx[:], dx[:])
                nc.scalar.square(dy[:], dy[:])
                d2 = d2p.tile([128, S], D2_DT, tag="d2", bufs=NQT)
                nc.vector.tensor_add(d2[:], dx[:], dy[:])
                return d2

            d2_pre = {qt: emit_d2(qt) for qt in range(NQT)}

            # All projection inputs are bf16 and load up front with no
            # SBUF-reuse wait. K/Q inputs live in a pool that closes before
            # the big attention pool opens (its range is reused); V inputs
            # stay live into q-tile 0, whose head loop interleaves the 16
            # deferred V projection chains.
            vp_ctx = tc.tile_pool(name="v_in", bufs=1)
            vp = vp_ctx.__enter__()
            vps_ctx = tc.tile_pool(name="vps", bufs=1, space="PSUM")
            vps = vps_ctx.__enter__()
            wvT_sb = vp.tile([128, NDT, D], BF16)
            xvT_sb = vp.tile([128, NDT, S], BF16)
            kq_ctx = tc.tile_pool(name="kq_in", bufs=1)
            kq = kq_ctx.__enter__()
            wkT_sb = kq.tile([128, NDT, D], BF16)
            xkT_sb = kq.tile([128, NDT, S], BF16)
            wqT_sb = kq.tile([128, NDT, D], BF16)
            xqT_sb = kq.tile([128, NDT, SQ], BF16)
            _load_chunked(nc, wkT_sb, wkT, D, D)
            _load_chunked(nc, xkT_sb, xkT, D, S)
            _load_chunked(nc, wqT_sb, wqT, D, D)
            _load_chunked(nc, xqT_sb, xqT, D, SQ)
            _load_chunked(nc, wvT_sb, wvT, D, D)
            _load_chunked(nc, xvT_sb, xvT, D, S)

            with tc.tile_pool(name="kps", bufs=4, space="PSUM") as kps:
                for m in range(NDT):
                    for hf in range(2):
                        ps = kps.tile([128, 512], F32, tag="pj")
                        for t in range(NDT):
                            nc.tensor.matmul(ps[:], wkT_sb[:, t, 128 * m:128 * m + 128],
                                             xkT_sb[:, t, 512 * hf:512 * hf + 512],
                                             start=(t == 0), stop=(t == NDT - 1))
                        nc.scalar.activation(kT[:, m, 512 * hf:512 * hf + 512], ps[:],
                                             ACT.Identity, bias=bk_col[:, m:m + 1])
                for m in range(NDT):
                    ps = kps.tile([128, 512], F32, tag="pj")
                    for t in range(NDT):
                        nc.tensor.matmul(ps[:], wqT_sb[:, t, 128 * m:128 * m + 128],
                                         xqT_sb[:, t, :],
                                         start=(t == 0), stop=(t == NDT - 1))
                    nc.scalar.activation(qT[:, m, :], ps[:], ACT.Identity,
                                         bias=bq_col[:, m:m + 1])

            kq_ctx.__exit__(None, None, None)
            # big attention pool opens after kq_in closed: reuses its range
            att_ctx = tc.tile_pool(name="att", bufs=2)
            att = att_ctx.__enter__()

            def emit_vchain(k):
                hf, m = k // NKT, k % NKT
                ps = vps.tile([128, 384], F32, tag="pj")
                for t in range(NDT):
                    nc.tensor.matmul(ps[:], xvT_sb[:, t, 128 * m:128 * m + 128],
                                     wvT_sb[:, t, 384 * hf:384 * hf + 384],
                                     start=(t == 0), stop=(t == NDT - 1))
                nc.scalar.copy(v_sb[:, m, 384 * hf:384 * hf + 384], ps[:])

            if stage == "proj":
                for k in range(2 * NKT):
                    emit_vchain(k)
                with tc.tile_pool(name="dump", bufs=1) as dp:
                    t = dp.tile([128, 512], F32)
                    nc.scalar.copy(t[:], qT[:, 0, :].bitcast(F32))
                    nc.sync.dma_start(out=out[0:128, 0:512], in_=t[:])
                    t2 = dp.tile([128, 512], F32)
                    nc.scalar.copy(t2[:], kT[:, 0, 0:512].bitcast(F32))
                    nc.sync.dma_start(out=out[128:256, 0:512], in_=t2[:])
                    t3 = dp.tile([128, 512], F32)
                    nc.vector.tensor_copy(t3[:], v_sb[:, 0, 0:512])
                    nc.sync.dma_start(out=out[256:384, 0:512], in_=t3[:])

            # ---- attention ----
            if stage != "proj":
              with tc.tile_pool(name="acc_dve", bufs=2) as acc_dve, \
                 tc.tile_pool(name="acc_pool", bufs=2) as acc_pool, \
                 tc.tile_pool(name="qe_ps", bufs=1, space="PSUM") as qe_ps, \
                 tc.tile_pool(name="qk_ps", bufs=2, space="PSUM") as qk_ps, \
                 tc.tile_pool(name="av_ps", bufs=1, space="PSUM") as av_ps, \
                 tc.tile_pool(name="pt_ps", bufs=1, space="PSUM") as pt_ps:
                # --- qe -> dqe for ALL q-tiles up front (right after the Q
                # projection) so no chain ever waits on an in-order-queued dqe
                # at a q-tile boundary. Block-diagonal emb: 2 heads per
                # matmul; 64-partition sliver matmuls proved flaky on HW. ---
                dqe_pre = {}
                for qt in range(NQT):
                    qe_psum = qe_ps.tile([128, H * NUM_EMB], F32, tag="qe")
                    for m in range(NDT):
                        nc.tensor.matmul(qe_psum[:, 20 * m:20 * m + 20],
                                         qT[:, m, 128 * qt:128 * qt + 128],
                                         embT_blk[:],
                                         start=True, stop=True)
                    qe_sb = d2p.tile([128, H, NUM_EMB], F32, tag="qe_sb", bufs=2)
                    nc.scalar.copy(qe_sb[:], qe_psum[:].rearrange("p (h e) -> p h e", e=NUM_EMB))
                    dqe = d2p.tile([128, H, NUM_EMB - 1], F32, tag="dqe", bufs=NQT)
                    nc.vector.tensor_tensor(out=dqe[:], in0=qe_sb[:, :, 1:],
                                            in1=qe_sb[:, :, :-1], op=ALU.subtract)
                    dqe_pre[qt] = dqe

                for qt in range(NQT):
                    d2 = d2_pre[qt]
                    dqe = dqe_pre[qt]
                    o_tile = att.tile([128, D], F32, tag="o_tile")
                    # Software-pipelined head loop. Per head the bias term is
                    # 9 masked-scaled rows t_e = (d2 >= T_e)*dqe[h,e], each a
                    # single fast-mode tensor_scalar on DVE. Accumulation of
                    # the t_e onto the qk logits happens per-head on one of:
                    #   - PE: identity-stationary matmuls accumulate the t_e
                    #     directly into the qk PSUM (most heads),
                    #   - DVE/Pool: tensor-tensor add chains in SBUF.
                    # PE-consuming stages are emitted LAG slots late so the
                    # in-order PE stream never waits on the current head.
                    lag = 4 if qt == 0 else 2
                    lag_f = lag + 1   # recip/final run one slot after AV
                    # binary step masks for this q-tile, shared by all heads
                    masks = att.tile([128, NUM_EMB - 1, S], BF16, tag="masks", bufs=2)
                    for e in range(NUM_EMB - 1):
                        nc.vector.tensor_scalar(out=masks[:, e, :], in0=d2[:],
                                                scalar1=THRESH2[e], scalar2=None,
                                                op0=ALU.is_ge)
                    tes = {}
                    state = {}
                    avs = {}
                    for slot in range(H + lag_f):
                        if slot < H:
                            h = slot
                            off = (64 * h) % 128
                            pe_acc = h not in DVE_CHAIN_HEADS and h not in POOL_CHAIN_HEADS
                            if pe_acc:
                                # per-head scaled rows t_e = dqe[h,e]*mask_e
                                # (4x-mode bf16 tensor_scalar on DVE)
                                te = att.tile([128, NUM_EMB - 1, S], BF16,
                                              tag="te", bufs=2)
                                for e in range(NUM_EMB - 1):
                                    nc.vector.tensor_scalar(
                                        out=te[:, e, :], in0=masks[:, e, :],
                                        scalar1=dqe[:, h, e:e + 1], scalar2=None,
                                        op0=ALU.mult)
                                tes[h] = te
                            else:
                                tes[h] = None
                            # --- logits = q.k^T (PE-acc heads leave the
                            # accumulation group open for the mask adds) ---
                            qk = qk_ps.tile([128, S], F32, tag="qk")
                            for hf in range(2):
                                nc.tensor.matmul(qk[:, 512 * hf:512 * hf + 512],
                                                 qT[off:off + 64, h // 2, 128 * qt:128 * qt + 128],
                                                 kT[off:off + 64, h // 2, 512 * hf:512 * hf + 512],
                                                 start=True, stop=not pe_acc)
                            if qt == 0 and slot < NKT:
                                emit_vchain(2 * slot)
                                emit_vchain(2 * slot + 1)
                            if not pe_acc:
                                # --- bias chain: per-e scaled mask row (DVE
                                # TS) + f32 TT add on the head's engine; the
                                # first add reads qk PSUM on DVE (Pool cannot
                                # access PSUM) ---
                                eng = nc.gpsimd if h in POOL_CHAIN_HEADS else nc.vector
                                accp = acc_pool if h in POOL_CHAIN_HEADS else acc_dve
                                src = qk
                                for e in range(NUM_EMB - 1):
                                    te1 = att.tile([128, S], BF16, tag="te1", bufs=2)
                                    nc.vector.tensor_scalar(
                                        out=te1[:], in0=masks[:, e, :],
                                        scalar1=dqe[:, h, e:e + 1], scalar2=None,
                                        op0=ALU.mult)
                                    acc = accp.tile([128, S], F32, tag="acc")
                                    op_eng = nc.vector if e == 0 else eng
                                    op_eng.tensor_tensor(out=acc[:], in0=te1[:],
                                                         in1=src[:], op=ALU.add)
                                    src = acc
                                state[h] = (src, None)
                            state.setdefault(h, (None, qk))
                        if 1 <= slot < H + 1:
                            # one slot after qk: PE accumulates dqe_e * mask_e
                            # onto qk via diag stationaries, then exp +
                            # row-sum + transpose
                            h1 = slot - 1
                            te = tes.pop(h1)
                            src, qk = state.pop(h1)
                            if src is None:
                                for e in range(NUM_EMB - 1):
                                    for hf in range(2):
                                        nc.tensor.matmul(
                                            qk[:, 512 * hf:512 * hf + 512],
                                            ident[:],
                                            te[:, e, 512 * hf:512 * hf + 512],
                                            start=False, stop=(e == NUM_EMB - 2))
                                src = qk
                            p_sb = att.tile([128, S], BF16, tag="p", bufs=3)
                            den = att.tile([128, 1], F32, tag="den", bufs=6)
                            nc.scalar.activation(p_sb[:], src[:], ACT.Exp, scale=0.125,
                                                 accum_out=den[:])
                            # P transpose on the PE (the xbar dma-transpose
                            # path intermittently corrupts pT on HW)
                            ptp = pt_ps.tile([128, NKT, 128], BF16, tag="ptp")
                            for c in range(NKT):
                                nc.tensor.transpose(ptp[:, c, :], p_sb[:, 128 * c:128 * c + 128], ident[:])
                            pT = att.tile([128, NKT, 128], BF16, tag="pT", bufs=5)
                            nc.scalar.copy(pT[:], ptp[:])
                            state[h1] = (pT, den)
                        if lag <= slot < H + lag:
                            h2 = slot - lag
                            pT, den = state.pop(h2)
                            # --- av = P^T . V_h ---
                            av = av_ps.tile([128, DK], F32, tag="av")
                            for c in range(NKT):
                                nc.tensor.matmul(av[:], pT[:, c, :],
                                                 v_sb[:, c, 64 * h2:64 * h2 + 64],
                                                 start=(c == 0), stop=(c == NKT - 1))
                            avs[h2] = (av, den)
                        if slot >= lag_f:
                            h3 = slot - lag_f
                            av, den = avs.pop(h3)
                            # --- out_h = av / den + bv_h ---
                            recip = att.tile([128, 1], F32, tag="recip")
                            nc.vector.reciprocal(recip[:], den[:])
                            nc.vector.scalar_tensor_tensor(
                                out=o_tile[:, 64 * h3:64 * h3 + 64], in0=av[:], scalar=recip[:],
                                in1=bv_b[:, 64 * h3:64 * h3 + 64], op0=ALU.mult, op1=ALU.add)
                    nc.sync.dma_start(out=out[128 * qt:128 * qt + 128, :], in_=o_tile[:])
            att_ctx.__exit__(None, None, None)
            vps_ctx.__exit__(None, None, None)
            vp_ctx.__exit__(None, None, None)
            mb_ctx.__exit__(None, None, None)
            d2p_ctx.__exit__(None, None, None)
    nc.compile()
    return nc


_NC_CACHE = {}


def _get_nc():
    if "nc" not in _NC_CACHE:
        _NC_CACHE["nc"] = build_nc()
    return _NC_CACHE["nc"]


def _make_in_maps(inputs):
    q = np.ascontiguousarray(np.asarray(inputs["query"], dtype=np.float32))
    k = np.ascontiguousarray(np.asarray(inputs["key"], dtype=np.float32))
    v = np.ascontiguousarray(np.asarray(inputs["value"], dtype=np.float32))
    tp = np.ascontiguousarray(np.asarray(inputs["tile_positions"], dtype=np.float32))
    import ml_dtypes
    WqT = np.ascontiguousarray(
        np.asarray(inputs["Wq"], dtype=np.float32).T.astype(ml_dtypes.bfloat16))
    WkT = np.ascontiguousarray(
        np.asarray(inputs["Wk"], dtype=np.float32).T.astype(ml_dtypes.bfloat16))
    WvT = np.ascontiguousarray(
        np.asarray(inputs["Wv"], dtype=np.float32).T.astype(ml_dtypes.bfloat16))
    bq = np.ascontiguousarray(np.asarray(inputs["bq"], dtype=np.float32))
    bk = np.ascontiguousarray(np.asarray(inputs["bk"], dtype=np.float32))
    bv = np.ascontiguousarray(np.asarray(inputs["bv"], dtype=np.float32))
    embT = np.ascontiguousarray(np.asarray(inputs["emb_k"], dtype=np.float32).T.astype(ml_dtypes.bfloat16))

    xqT_b = [np.ascontiguousarray(q[b].T.astype(ml_dtypes.bfloat16)) for b in range(B)]
    xkT_b = [np.ascontiguousarray(k[b].T.astype(ml_dtypes.bfloat16)) for b in range(B)]
    xvT_b = [np.ascontiguousarray(v[b].T.astype(ml_dtypes.bfloat16)) for b in range(B)]

    in_maps = []
    for c in range(NCORES):
        b, qh = c // 2, c % 2
        in_maps.append({
            "xqT": np.ascontiguousarray(xqT_b[b][:, qh * SQ:(qh + 1) * SQ]),
            "xkT": xkT_b[b], "xvT": xvT_b[b],
            "pos": tp[b],
            "posq": np.ascontiguousarray(tp[b, qh * SQ:(qh + 1) * SQ]),
            "wqT": WqT, "wkT": WkT, "wvT": WvT,
            "bq": bq, "bk": bk, "bv": bv,
            "embT_h": embT,
        })
    return in_maps


def kernel(query, key, value, tile_positions, Wq, bq, Wk, bk, Wv, bv, emb_k):
    inputs = {"query": query, "key": key, "value": value,
              "tile_positions": tile_positions,
              "Wq": Wq, "bq": bq, "Wk": Wk, "bk": bk, "Wv": Wv, "bv": bv,
              "emb_k": emb_k}
    nc = _get_nc()
    in_maps = _make_in_maps(inputs)
    res = run_bass_kernel_spmd(nc, in_maps, core_ids=list(range(NCORES)))
    out = np.empty((B, S, D), np.float32)
    for c in range(NCORES):
        b, qh = c // 2, c % 2
        out[b, qh * SQ:(qh + 1) * SQ] = res.results[c]["out"]
    return out


# revision 50
# speedup vs baseline: 14389.1449x; 1.2782x over previous
"""Distance-aware multihead attention on 8 Trainium2 NeuronCores.

Problem: B=4, S=1024, D=768, H=12, DK=64, NUM_EMB=10.
  q/k/v = linear projections of query/key/value
  idx[b,i,j] = clip(round(9 * |pos_i - pos_j| / MAXD), 0, 9)
  logits = (q.k^T + qe[b,h,i,idx[b,i,j]]) / 8   where qe = q @ emb_k^T
  out = softmax(logits) @ v

Key decompositions:
  - bias qe[...,idx] = qe[...,0] + sum_{e=1..9} (qe_e - qe_{e-1}) * (d2 >= T_e^2);
    the qe_0 term is constant along the softmax axis and cancels -> dropped.
  - step masks (d2 >= T_e^2) are shared across all 12 heads of a q-tile.
  - bias accumulated onto QK logits via 9 scalar_tensor_tensor ops per (head,
    q-tile); the 12 head-chains are split across the DVE and Pool engines.

Sharding: core c handles batch c//2, query-half c%2 (512 queries, all heads).
K/V/projections are computed per-core from full-S inputs (duplicated across the
2 cores sharing a batch); masks/logits/AV are not duplicated.

Layouts: all matmul operands are loaded pre-transposed from DRAM (the host
transposes W/X/emb in kernel(), so every DMA is a wide contiguous copy — the
4-byte-granular xbar/broadcast DMA patterns this replaced were ~30x slower).
Q^T/K^T [dim, token] f32r, V [token, dim] bf16.
P = exp((qk+bias)/8) bf16 in [q, k]; transposed to [k, q] 128-chunks on the PE;
AV accumulates over the 8 k-chunks on TensorE.
"""
import os
import numpy as np

import concourse.bass as bass
import concourse.tile as tile
from concourse import bacc, mybir
from concourse.bass_utils import run_bass_kernel_spmd

F32 = mybir.dt.float32
F32R = mybir.dt.float32r
BF16 = mybir.dt.bfloat16
ACT = mybir.ActivationFunctionType
ALU = mybir.AluOpType

B, S, D = 4, 1024, 768
H, DK = 12, 64
NUM_EMB = 10
MAX_DIST = 100000.0 * 2 ** 0.5
SQ = S // 2          # queries per core
NQT = SQ // 128      # q-tiles per core (4)
NKT = S // 128       # k token chunks (8)
NDT = D // 128       # dim tiles (6)
NCORES = 8

# squared thresholds: idx >= e  <=>  d2 >= ((e-0.5)*MAX_DIST/9)^2
THRESH2 = [float(((e - 0.5) * MAX_DIST / 9.0) ** 2) for e in range(1, NUM_EMB)]

# bias-accumulation engine per head: most heads accumulate the step masks
# onto the qk PSUM via diag(dqe)-stationary matmuls on the PE; the heads
# listed here instead run f32 tensor-tensor add chains on DVE / Pool to
# balance engine load.
DVE_CHAIN_HEADS = frozenset(
    int(h) for h in os.environ.get("DVE_CHAIN_HEADS", "").split(",") if h != "")
POOL_CHAIN_HEADS = frozenset(
    int(h) for h in os.environ.get("POOL_CHAIN_HEADS", "").split(",") if h != "")
# d2 stays f32: bf16 d2 misbuckets enough pairs to breach tolerance on HW
D2_DT = mybir.dt.float32


def _load_chunked(nc, dst, src_dram, rows, cols):
    """src [rows, cols] DRAM -> dst [128, rows//128, cols] SBUF with
    dst[p, g, c] = src[128*g + p, c]. One wide contiguous DMA."""
    g = rows // 128
    nc.sync.dma_start(
        out=dst[:],
        in_=bass.AP(tensor=src_dram.tensor, offset=0,
                    ap=[[cols, 128], [128 * cols, g], [1, cols]]))


def build_nc(stage="full"):
    nc = bacc.Bacc("TRN2", target_bir_lowering=False, debug=False)

    # matmul-feeding inputs are float32r so the fp32r verifier accepts
    # DMA -> SBUF -> matmul (host values are plain fp32 bits).
    xqT = nc.dram_tensor("xqT", [D, SQ], BF16, kind="ExternalInput").ap()
    xkT = nc.dram_tensor("xkT", [D, S], BF16, kind="ExternalInput").ap()
    xvT = nc.dram_tensor("xvT", [D, S], BF16, kind="ExternalInput").ap()
    pos = nc.dram_tensor("pos", [S, 2], F32, kind="ExternalInput").ap()
    posq = nc.dram_tensor("posq", [SQ, 2], F32, kind="ExternalInput").ap()
    wqT = nc.dram_tensor("wqT", [D, D], BF16, kind="ExternalInput").ap()
    wkT = nc.dram_tensor("wkT", [D, D], BF16, kind="ExternalInput").ap()
    wvT = nc.dram_tensor("wvT", [D, D], BF16, kind="ExternalInput").ap()
    bq = nc.dram_tensor("bq", [D], F32, kind="ExternalInput").ap()
    bk = nc.dram_tensor("bk", [D], F32, kind="ExternalInput").ap()
    bv = nc.dram_tensor("bv", [D], F32, kind="ExternalInput").ap()
    embT_h = nc.dram_tensor("embT_h", [DK, NUM_EMB], BF16, kind="ExternalInput").ap()
    out = nc.dram_tensor("out", [SQ, D], F32, kind="ExternalOutput").ap()

    with tile.TileContext(nc) as tc:
        with tc.tile_pool(name="persist", bufs=1) as persist:
            # ---- setup: bias columns, position broadcasts ----
            bq_col = persist.tile([128, NDT], F32)
            bk_col = persist.tile([128, NDT], F32)
            nc.sync.dma_start(out=bq_col[:], in_=bass.AP(tensor=bq.tensor, offset=0, ap=[[1, 128], [128, NDT]]))
            nc.sync.dma_start(out=bk_col[:], in_=bass.AP(tensor=bk.tensor, offset=0, ap=[[1, 128], [128, NDT]]))
            bv_b = persist.tile([128, D], F32)
            nc.sync.dma_start(out=bv_b[:], in_=bass.AP(tensor=bv.tensor, offset=0, ap=[[0, 128], [1, D]]))
            # key positions: load rows on partition 0, broadcast on Pool
            xrow = persist.tile([1, S], F32)
            yrow = persist.tile([1, S], F32)
            nc.sync.dma_start(out=xrow[:], in_=bass.AP(tensor=pos.tensor, offset=0, ap=[[2048, 1], [2, S]]))
            nc.sync.dma_start(out=yrow[:], in_=bass.AP(tensor=pos.tensor, offset=1, ap=[[2048, 1], [2, S]]))
            xk_b = persist.tile([128, S], F32)
            yk_b = persist.tile([128, S], F32)
            nc.gpsimd.partition_broadcast(xk_b[:], xrow[:])
            nc.gpsimd.partition_broadcast(yk_b[:], yrow[:])
            # query positions as per-partition scalars [128, NQT]
            xq_col = persist.tile([128, NQT], F32)
            yq_col = persist.tile([128, NQT], F32)
            nc.sync.dma_start(out=xq_col[:], in_=bass.AP(tensor=posq.tensor, offset=0, ap=[[2, 128], [256, NQT]]))
            nc.sync.dma_start(out=yq_col[:], in_=bass.AP(tensor=posq.tensor, offset=1, ap=[[2, 128], [256, NQT]]))
            # emb^T block-diagonal [128, 20]: [0:64,0:10] and [64:128,10:20]
            embT_blk = persist.tile([128, 2 * NUM_EMB], BF16)
            nc.vector.memset(embT_blk[:], 0.0)
            nc.sync.dma_start(out=embT_blk[0:64, 0:NUM_EMB],
                              in_=bass.AP(tensor=embT_h.tensor, offset=0, ap=[[NUM_EMB, 64], [1, NUM_EMB]]))
            nc.sync.dma_start(out=embT_blk[64:128, NUM_EMB:2 * NUM_EMB],
                              in_=bass.AP(tensor=embT_h.tensor, offset=0, ap=[[NUM_EMB, 64], [1, NUM_EMB]]))

            ident = persist.tile([128, 128], BF16)
            from concourse.masks import make_identity
            make_identity(nc, ident[:])
            # dummy 2-byte xbar transpose: the first xbar op in a kernel can
            # glitch (mode transition); this one absorbs it.
            scrap = persist.tile([128, 128], BF16)
            scrapT = persist.tile([128, 128], BF16)
            nc.vector.memset(scrap[:], 0.0)
            nc.sync.dma_start_transpose(scrapT[:], scrap[:])
            v_sb = persist.tile([128, NKT, D], BF16)   # V[token, dim], token-chunked
            kT = persist.tile([128, NDT, S], BF16)     # K^T[dim, token]
            qT = persist.tile([128, NDT, SQ], BF16)    # Q^T[dim, token]

            # ---- d2 prebuild ----
            # a small early pool so all four q-tiles' squared-distance rows
            # build on the (otherwise idle) DVE during the projection phase
            d2p_ctx = tc.tile_pool(name="d2p", bufs=1)
            d2p = d2p_ctx.__enter__()
            mb_ctx = tc.tile_pool(name="maskbuild", bufs=1)
            mb = mb_ctx.__enter__()

            def emit_d2(qt):
                dx = mb.tile([128, S], F32, tag="dx")
                dy = mb.tile([128, S], F32, tag="dy")
                nc.vector.tensor_scalar(out=dx[:], in0=xk_b[:], scalar1=xq_col[:, qt:qt + 1],
                                        scalar2=None, op0=ALU.subtract)
                nc.vector.tensor_scalar(out=dy[:], in0=yk_b[:], scalar1=yq_col[:, qt:qt + 1],
                                        scalar2=None, op0=ALU.subtract)
                nc.scalar.square(dx[:], dx[:])
                nc.scalar.square(dy[:], dy[:])
                d2 = d2p.tile([128, S], D2_DT, tag="d2", bufs=NQT)
                nc.vector.tensor_add(d2[:], dx[:], dy[:])
                return d2

            d2_pre = {qt: emit_d2(qt) for qt in range(NQT)}

            # All projection inputs are bf16 and load up front with no
            # SBUF-reuse wait. K/Q inputs live in a pool that closes before
            # the big attention pool opens (its range is reused); V inputs
            # stay live into q-tile 0, whose head loop interleaves the 16
            # deferred V projection chains.
            vp_ctx = tc.tile_pool(name="v_in", bufs=1)
            vp = vp_ctx.__enter__()
            vps_ctx = tc.tile_pool(name="vps", bufs=1, space="PSUM")
            vps = vps_ctx.__enter__()
            wvT_sb = vp.tile([128, NDT, D], BF16)
            xvT_sb = vp.tile([128, NDT, S], BF16)
            kq_ctx = tc.tile_pool(name="kq_in", bufs=1)
            kq = kq_ctx.__enter__()
            wkT_sb = kq.tile([128, NDT, D], BF16)
            xkT_sb = kq.tile([128, NDT, S], BF16)
            wqT_sb = kq.tile([128, NDT, D], BF16)
            xqT_sb = kq.tile([128, NDT, SQ], BF16)
            _load_chunked(nc, wkT_sb, wkT, D, D)
            _load_chunked(nc, xkT_sb, xkT, D, S)
            _load_chunked(nc, wqT_sb, wqT, D, D)
            _load_chunked(nc, xqT_sb, xqT, D, SQ)
            _load_chunked(nc, wvT_sb, wvT, D, D)
            _load_chunked(nc, xvT_sb, xvT, D, S)

            with tc.tile_pool(name="kps", bufs=4, space="PSUM") as kps:
                for m in range(NDT):
                    for hf in range(2):
                        ps = kps.tile([128, 512], F32, tag="pj")
                        for t in range(NDT):
                            nc.tensor.matmul(ps[:], wkT_sb[:, t, 128 * m:128 * m + 128],
                                             xkT_sb[:, t, 512 * hf:512 * hf + 512],
                                             start=(t == 0), stop=(t == NDT - 1))
                        nc.scalar.activation(kT[:, m, 512 * hf:512 * hf + 512], ps[:],
                                             ACT.Identity, bias=bk_col[:, m:m + 1])
                for m in range(NDT):
                    ps = kps.tile([128, 512], F32, tag="pj")
                    for t in range(NDT):
                        nc.tensor.matmul(ps[:], wqT_sb[:, t, 128 * m:128 * m + 128],
                                         xqT_sb[:, t, :],
                                         start=(t == 0), stop=(t == NDT - 1))
                    nc.scalar.activation(qT[:, m, :], ps[:], ACT.Identity,
                                         bias=bq_col[:, m:m + 1])

            kq_ctx.__exit__(None, None, None)
            # big attention pool opens after kq_in closed: reuses its range
            att_ctx = tc.tile_pool(name="att", bufs=2)
            att = att_ctx.__enter__()

            def emit_vchain(k):
                hf, m = k // NKT, k % NKT
                ps = vps.tile([128, 384], F32, tag="pj")
                for t in range(NDT):
                    nc.tensor.matmul(ps[:], xvT_sb[:, t, 128 * m:128 * m + 128],
                                     wvT_sb[:, t, 384 * hf:384 * hf + 384],
                                     start=(t == 0), stop=(t == NDT - 1))
                nc.scalar.copy(v_sb[:, m, 384 * hf:384 * hf + 384], ps[:])

            if stage == "proj":
                for k in range(2 * NKT):
                    emit_vchain(k)
                with tc.tile_pool(name="dump", bufs=1) as dp:
                    t = dp.tile([128, 512], F32)
                    nc.scalar.copy(t[:], qT[:, 0, :].bitcast(F32))
                    nc.sync.dma_start(out=out[0:128, 0:512], in_=t[:])
                    t2 = dp.tile([128, 512], F32)
                    nc.scalar.copy(t2[:], kT[:, 0, 0:512].bitcast(F32))
                    nc.sync.dma_start(out=out[128:256, 0:512], in_=t2[:])
                    t3 = dp.tile([128, 512], F32)
                    nc.vector.tensor_copy(t3[:], v_sb[:, 0, 0:512])
                    nc.sync.dma_start(out=out[256:384, 0:512], in_=t3[:])

            # ---- attention ----
            if stage != "proj":
              with tc.tile_pool(name="acc_dve", bufs=2) as acc_dve, \
                 tc.tile_pool(name="acc_pool", bufs=2) as acc_pool, \
                 tc.tile_pool(name="qe_ps", bufs=1, space="PSUM") as qe_ps, \
                 tc.tile_pool(name="qk_ps", bufs=2, space="PSUM") as qk_ps, \
                 tc.tile_pool(name="av_ps", bufs=1, space="PSUM") as av_ps, \
                 tc.tile_pool(name="pt_ps", bufs=1, space="PSUM") as pt_ps:
                # --- qe -> dqe for ALL q-tiles up front (right after the Q
                # projection) so no chain ever waits on an in-order-queued dqe
                # at a q-tile boundary. Block-diagonal emb: 2 heads per
                # matmul; 64-partition sliver matmuls proved flaky on HW. ---
                dqe_pre = {}
                for qt in range(NQT):
                    qe_psum = qe_ps.tile([128, H * NUM_EMB], F32, tag="qe")
                    for m in range(NDT):
                        nc.tensor.matmul(qe_psum[:, 20 * m:20 * m + 20],
                                         qT[:, m, 128 * qt:128 * qt + 128],
                                         embT_blk[:],
                                         start=True, stop=True)
                    qe_sb = d2p.tile([128, H, NUM_EMB], F32, tag="qe_sb", bufs=2)
                    nc.scalar.copy(qe_sb[:], qe_psum[:].rearrange("p (h e) -> p h e", e=NUM_EMB))
                    dqe = d2p.tile([128, H, NUM_EMB - 1], F32, tag="dqe", bufs=NQT)
                    nc.vector.tensor_tensor(out=dqe[:], in0=qe_sb[:, :, 1:],
                                            in1=qe_sb[:, :, :-1], op=ALU.subtract)
                    dqe_pre[qt] = dqe

                for qt in range(NQT):
                    d2 = d2_pre[qt]
                    dqe = dqe_pre[qt]
                    o_tile = att.tile([128, D], F32, tag="o_tile")
                    # Software-pipelined head loop. Per head the bias term is
                    # 9 masked-scaled rows t_e = (d2 >= T_e)*dqe[h,e], each a
                    # single fast-mode tensor_scalar on DVE. Accumulation of
                    # the t_e onto the qk logits happens per-head on one of:
                    #   - PE: identity-stationary matmuls accumulate the t_e
                    #     directly into the qk PSUM (most heads),
                    #   - DVE/Pool: tensor-tensor add chains in SBUF.
                    # PE-consuming stages are emitted LAG slots late so the
                    # in-order PE stream never waits on the current head.
                    lag = 4 if qt == 0 else 2
                    lag_f = lag + 1   # recip/final run one slot after AV
                    # binary step masks for this q-tile, shared by all heads
                    masks = att.tile([128, NUM_EMB - 1, S], BF16, tag="masks", bufs=2)
                    for e in range(NUM_EMB - 1):
                        nc.vector.tensor_scalar(out=masks[:, e, :], in0=d2[:],
                                                scalar1=THRESH2[e], scalar2=None,
                                                op0=ALU.is_ge)
                    tes = {}
                    state = {}
                    avs = {}
                    for slot in range(H + lag_f):
                        if slot < H:
                            h = slot
                            off = (64 * h) % 128
                            pe_acc = h not in DVE_CHAIN_HEADS and h not in POOL_CHAIN_HEADS
                            if pe_acc:
                                # per-head scaled rows t_e = dqe[h,e]*mask_e
                                # (4x-mode bf16 tensor_scalar on DVE)
                                te = att.tile([128, NUM_EMB - 1, S], BF16,
                                              tag="te", bufs=2)
                                for e in range(NUM_EMB - 1):
                                    nc.vector.tensor_scalar(
                                        out=te[:, e, :], in0=masks[:, e, :],
                                        scalar1=dqe[:, h, e:e + 1], scalar2=None,
                                        op0=ALU.mult)
                            else:
                                te = None
                            tes[h] = te
                            # --- logits = q.k^T (PE-acc heads leave the
                            # accumulation group open for the mask adds) ---
                            qk = qk_ps.tile([128, S], F32, tag="qk")
                            for hf in range(2):
                                nc.tensor.matmul(qk[:, 512 * hf:512 * hf + 512],
                                                 qT[off:off + 64, h // 2, 128 * qt:128 * qt + 128],
                                                 kT[off:off + 64, h // 2, 512 * hf:512 * hf + 512],
                                                 start=True, stop=not pe_acc)
                            if qt == 0 and slot < NKT:
                                emit_vchain(2 * slot)
                                emit_vchain(2 * slot + 1)
                            if not pe_acc:
                                # --- bias via 9 chained masked MACs on DVE
                                # (the original scalar_tensor_tensor form) ---
                                src = qk
                                for e in range(NUM_EMB - 1):
                                    acc = acc_dve.tile([128, S], F32, tag="acc")
                                    nc.vector.scalar_tensor_tensor(
                                        out=acc[:], in0=masks[:, e, :],
                                        scalar=dqe[:, h, e:e + 1],
                                        in1=src[:], op0=ALU.mult, op1=ALU.add)
                                    src = acc
                                state[h] = (src, None)
                            state.setdefault(h, (None, qk))
                        if 1 <= slot < H + 1:
                            # one slot after qk: PE accumulates dqe_e * mask_e
                            # onto qk via diag stationaries, then exp +
                            # row-sum + transpose
                            h1 = slot - 1
                            te = tes.pop(h1)
                            src, qk = state.pop(h1)
                            if src is None:
                                for e in range(NUM_EMB - 1):
                                    for hf in range(2):
                                        nc.tensor.matmul(
                                            qk[:, 512 * hf:512 * hf + 512],
                                            ident[:],
                                            te[:, e, 512 * hf:512 * hf + 512],
                                            start=False, stop=(e == NUM_EMB - 2))
                                src = qk
                            p_sb = att.tile([128, S], BF16, tag="p", bufs=3)
                            den = att.tile([128, 1], F32, tag="den", bufs=6)
                            nc.scalar.activation(p_sb[:], src[:], ACT.Exp, scale=0.125,
                                                 accum_out=den[:])
                            # P transpose on the PE (the xbar dma-transpose
                            # path intermittently corrupts pT on HW)
                            ptp = pt_ps.tile([128, NKT, 128], BF16, tag="ptp")
                            for c in range(NKT):
                                nc.tensor.transpose(ptp[:, c, :], p_sb[:, 128 * c:128 * c + 128], ident[:])
                            pT = att.tile([128, NKT, 128], BF16, tag="pT", bufs=5)
                            nc.scalar.copy(pT[:], ptp[:])
                            state[h1] = (pT, den)
                        if lag <= slot < H + lag:
                            h2 = slot - lag
                            pT, den = state.pop(h2)
                            # --- av = P^T . V_h ---
                            av = av_ps.tile([128, DK], F32, tag="av")
                            for c in range(NKT):
                                nc.tensor.matmul(av[:], pT[:, c, :],
                                                 v_sb[:, c, 64 * h2:64 * h2 + 64],
                                                 start=(c == 0), stop=(c == NKT - 1))
                            avs[h2] = (av, den)
                        if slot >= lag_f:
                            h3 = slot - lag_f
                            av, den = avs.pop(h3)
                            # --- out_h = av / den + bv_h ---
                            recip = att.tile([128, 1], F32, tag="recip")
                            nc.vector.reciprocal(recip[:], den[:])
                            nc.vector.scalar_tensor_tensor(
                                out=o_tile[:, 64 * h3:64 * h3 + 64], in0=av[:], scalar=recip[:],
                                in1=bv_b[:, 64 * h3:64 * h3 + 64], op0=ALU.mult, op1=ALU.add)
                    nc.sync.dma_start(out=out[128 * qt:128 * qt + 128, :], in_=o_tile[:])
            att_ctx.__exit__(None, None, None)
            vps_ctx.__exit__(None, None, None)
            vp_ctx.__exit__(None, None, None)
            mb_ctx.__exit__(None, None, None)
            d2p_ctx.__exit__(None, None, None)
    nc.compile()
    return nc


_NC_CACHE = {}


def _get_nc():
    if "nc" not in _NC_CACHE:
        _NC_CACHE["nc"] = build_nc()
    return _NC_CACHE["nc"]


def _make_in_maps(inputs):
    q = np.ascontiguousarray(np.asarray(inputs["query"], dtype=np.float32))
    k = np.ascontiguousarray(np.asarray(inputs["key"], dtype=np.float32))
    v = np.ascontiguousarray(np.asarray(inputs["value"], dtype=np.float32))
    tp = np.ascontiguousarray(np.asarray(inputs["tile_positions"], dtype=np.float32))
    import ml_dtypes
    WqT = np.ascontiguousarray(
        np.asarray(inputs["Wq"], dtype=np.float32).T.astype(ml_dtypes.bfloat16))
    WkT = np.ascontiguousarray(
        np.asarray(inputs["Wk"], dtype=np.float32).T.astype(ml_dtypes.bfloat16))
    WvT = np.ascontiguousarray(
        np.asarray(inputs["Wv"], dtype=np.float32).T.astype(ml_dtypes.bfloat16))
    bq = np.ascontiguousarray(np.asarray(inputs["bq"], dtype=np.float32))
    bk = np.ascontiguousarray(np.asarray(inputs["bk"], dtype=np.float32))
    bv = np.ascontiguousarray(np.asarray(inputs["bv"], dtype=np.float32))
    embT = np.ascontiguousarray(np.asarray(inputs["emb_k"], dtype=np.float32).T.astype(ml_dtypes.bfloat16))

    xqT_b = [np.ascontiguousarray(q[b].T.astype(ml_dtypes.bfloat16)) for b in range(B)]
    xkT_b = [np.ascontiguousarray(k[b].T.astype(ml_dtypes.bfloat16)) for b in range(B)]
    xvT_b = [np.ascontiguousarray(v[b].T.astype(ml_dtypes.bfloat16)) for b in range(B)]

    in_maps = []
    for c in range(NCORES):
        b, qh = c // 2, c % 2
        in_maps.append({
            "xqT": np.ascontiguousarray(xqT_b[b][:, qh * SQ:(qh + 1) * SQ]),
            "xkT": xkT_b[b], "xvT": xvT_b[b],
            "pos": tp[b],
            "posq": np.ascontiguousarray(tp[b, qh * SQ:(qh + 1) * SQ]),
            "wqT": WqT, "wkT": WkT, "wvT": WvT,
            "bq": bq, "bk": bk, "bv": bv,
            "embT_h": embT,
        })
    return in_maps


def kernel(query, key, value, tile_positions, Wq, bq, Wk, bk, Wv, bv, emb_k):
    inputs = {"query": query, "key": key, "value": value,
              "tile_positions": tile_positions,
              "Wq": Wq, "bq": bq, "Wk": Wk, "bk": bk, "Wv": Wv, "bv": bv,
              "emb_k": emb_k}
    nc = _get_nc()
    in_maps = _make_in_maps(inputs)
    res = run_bass_kernel_spmd(nc, in_maps, core_ids=list(range(NCORES)))
    out = np.empty((B, S, D), np.float32)
    for c in range(NCORES):
        b, qh = c // 2, c % 2
        out[b, qh * SQ:(qh + 1) * SQ] = res.results[c]["out"]
    return out


# revision 62
# speedup vs baseline: 14766.3882x; 1.0262x over previous
"""Distance-aware multihead attention on 8 Trainium2 NeuronCores.

Problem: B=4, S=1024, D=768, H=12, DK=64, NUM_EMB=10.
  q/k/v = linear projections of query/key/value
  idx[b,i,j] = clip(round(9 * |pos_i - pos_j| / MAXD), 0, 9)
  logits = (q.k^T + qe[b,h,i,idx[b,i,j]]) / 8   where qe = q @ emb_k^T
  out = softmax(logits) @ v

Key decompositions:
  - bias qe[...,idx] = qe[...,0] + sum_{e=1..9} (qe_e - qe_{e-1}) * (d2 >= T_e^2);
    the qe_0 term is constant along the softmax axis and cancels -> dropped.
  - binary step masks (d2 >= T_e^2) are built once per q-tile, shared by all
    12 heads; per-head scaled rows t_e = dqe[h,e]*mask_e are one 4x-mode bf16
    tensor_scalar each on DVE.
  - the t_e are ADDED onto the open qk PSUM accumulation group by
    identity-stationary matmuls on the otherwise idle TensorE (most heads);
    one head per q-tile keeps the original DVE scalar_tensor_tensor chain to
    balance engine load.
  - the head loop is software-pipelined (qk at slot h; accumulate+exp+
    transpose at h+1; AV at h+lag; recip/final at h+lag+1) so the in-order PE
    stream never waits on the head currently being bias-accumulated.

Sharding: core c handles batch c//2, query-half c%2 (512 queries, all heads).
K/V/projections are computed per-core from full-S inputs (duplicated across the
2 cores sharing a batch); masks/logits/AV are not duplicated.

Layouts: all inputs are host-transposed and bf16, so every DMA is a wide
contiguous copy (the 4-byte-granular xbar/broadcast DMA patterns this replaced
were ~30x slower). d2 for all 4 q-tiles and qe->dqe prebuild during the
projection phase; the V projection's 16 psum chains are interleaved into
q-tile 0's head loop. P = exp((qk+bias)/8) bf16, transposed on the PE (the
xbar dma-transpose intermittently corrupts pT on HW); AV accumulates over the
8 k-chunks on TensorE.

Timeline-sim (cost model) per-core time: ~307 us vs ~2,419 us for the
original STT-only DMA-transpose-loading baseline (7.9x). Small setup
DMAs issue from the Act queue so the critical K/Q/V input loads lead the
SP queue at t=0.
"""
import os
import numpy as np

import concourse.bass as bass
import concourse.tile as tile
from concourse import bacc, mybir
from concourse.bass_utils import run_bass_kernel_spmd

F32 = mybir.dt.float32
F32R = mybir.dt.float32r
BF16 = mybir.dt.bfloat16
ACT = mybir.ActivationFunctionType
ALU = mybir.AluOpType

B, S, D = 4, 1024, 768
H, DK = 12, 64
NUM_EMB = 10
MAX_DIST = 100000.0 * 2 ** 0.5
SQ = S // 2          # queries per core
NQT = SQ // 128      # q-tiles per core (4)
NKT = S // 128       # k token chunks (8)
NDT = D // 128       # dim tiles (6)
NCORES = 8

# squared thresholds: idx >= e  <=>  d2 >= ((e-0.5)*MAX_DIST/9)^2
THRESH2 = [float(((e - 0.5) * MAX_DIST / 9.0) ** 2) for e in range(1, NUM_EMB)]

# bias-accumulation engine per head: most heads accumulate the scaled step
# masks onto the qk PSUM via identity-stationary matmuls on the PE; the heads
# listed here instead run the original scalar_tensor_tensor chains on DVE
# (POOL_CHAIN_HEADS kept for experiments; Pool TT-adds proved too slow).
DVE_CHAIN_HEADS = frozenset(
    int(h) for h in os.environ.get("DVE_CHAIN_HEADS", "5").split(",") if h != "")
POOL_CHAIN_HEADS = frozenset(
    int(h) for h in os.environ.get("POOL_CHAIN_HEADS", "").split(",") if h != "")
# d2 stays f32: bf16 d2 misbuckets enough pairs to breach tolerance on HW
D2_DT = mybir.dt.float32


def _load_chunked(nc, dst, src_dram, rows, cols):
    """src [rows, cols] DRAM -> dst [128, rows//128, cols] SBUF with
    dst[p, g, c] = src[128*g + p, c]. One wide contiguous DMA."""
    g = rows // 128
    nc.sync.dma_start(
        out=dst[:],
        in_=bass.AP(tensor=src_dram.tensor, offset=0,
                    ap=[[cols, 128], [128 * cols, g], [1, cols]]))


def build_nc(stage="full"):
    nc = bacc.Bacc("TRN2", target_bir_lowering=False, debug=False)

    # matmul-feeding inputs are float32r so the fp32r verifier accepts
    # DMA -> SBUF -> matmul (host values are plain fp32 bits).
    xqT = nc.dram_tensor("xqT", [D, SQ], BF16, kind="ExternalInput").ap()
    xkT = nc.dram_tensor("xkT", [D, S], BF16, kind="ExternalInput").ap()
    xvT = nc.dram_tensor("xvT", [D, S], BF16, kind="ExternalInput").ap()
    pos = nc.dram_tensor("pos", [S, 2], F32, kind="ExternalInput").ap()
    posq = nc.dram_tensor("posq", [SQ, 2], F32, kind="ExternalInput").ap()
    wqT = nc.dram_tensor("wqT", [D, D], BF16, kind="ExternalInput").ap()
    wkT = nc.dram_tensor("wkT", [D, D], BF16, kind="ExternalInput").ap()
    wvT = nc.dram_tensor("wvT", [D, D], BF16, kind="ExternalInput").ap()
    bq = nc.dram_tensor("bq", [D], F32, kind="ExternalInput").ap()
    bk = nc.dram_tensor("bk", [D], F32, kind="ExternalInput").ap()
    bv = nc.dram_tensor("bv", [D], F32, kind="ExternalInput").ap()
    embT_h = nc.dram_tensor("embT_h", [DK, NUM_EMB], BF16, kind="ExternalInput").ap()
    out = nc.dram_tensor("out", [SQ, D], F32, kind="ExternalOutput").ap()

    with tile.TileContext(nc) as tc:
        with tc.tile_pool(name="persist", bufs=1) as persist:
            # ---- setup: bias columns, position broadcasts ----
            bq_col = persist.tile([128, NDT], F32)
            bk_col = persist.tile([128, NDT], F32)
            nc.scalar.dma_start(out=bq_col[:], in_=bass.AP(tensor=bq.tensor, offset=0, ap=[[1, 128], [128, NDT]]))
            nc.scalar.dma_start(out=bk_col[:], in_=bass.AP(tensor=bk.tensor, offset=0, ap=[[1, 128], [128, NDT]]))
            bv_b = persist.tile([128, D], F32)
            nc.scalar.dma_start(out=bv_b[:], in_=bass.AP(tensor=bv.tensor, offset=0, ap=[[0, 128], [1, D]]))
            # key positions: load rows on partition 0, broadcast on Pool
            xrow = persist.tile([1, S], F32)
            yrow = persist.tile([1, S], F32)
            nc.scalar.dma_start(out=xrow[:], in_=bass.AP(tensor=pos.tensor, offset=0, ap=[[2048, 1], [2, S]]))
            nc.scalar.dma_start(out=yrow[:], in_=bass.AP(tensor=pos.tensor, offset=1, ap=[[2048, 1], [2, S]]))
            xk_b = persist.tile([128, S], F32)
            yk_b = persist.tile([128, S], F32)
            nc.gpsimd.partition_broadcast(xk_b[:], xrow[:])
            nc.gpsimd.partition_broadcast(yk_b[:], yrow[:])
            # query positions as per-partition scalars [128, NQT]
            xq_col = persist.tile([128, NQT], F32)
            yq_col = persist.tile([128, NQT], F32)
            nc.scalar.dma_start(out=xq_col[:], in_=bass.AP(tensor=posq.tensor, offset=0, ap=[[2, 128], [256, NQT]]))
            nc.scalar.dma_start(out=yq_col[:], in_=bass.AP(tensor=posq.tensor, offset=1, ap=[[2, 128], [256, NQT]]))
            # emb^T block-diagonal [128, 20]: [0:64,0:10] and [64:128,10:20]
            embT_blk = persist.tile([128, 2 * NUM_EMB], BF16)
            nc.vector.memset(embT_blk[:], 0.0)
            nc.scalar.dma_start(out=embT_blk[0:64, 0:NUM_EMB],
                              in_=bass.AP(tensor=embT_h.tensor, offset=0, ap=[[NUM_EMB, 64], [1, NUM_EMB]]))
            nc.scalar.dma_start(out=embT_blk[64:128, NUM_EMB:2 * NUM_EMB],
                              in_=bass.AP(tensor=embT_h.tensor, offset=0, ap=[[NUM_EMB, 64], [1, NUM_EMB]]))

            ident = persist.tile([128, 128], BF16)
            from concourse.masks import make_identity
            make_identity(nc, ident[:])
            # dummy 2-byte xbar transpose: the first xbar op in a kernel can
            # glitch (mode transition); this one absorbs it.
            scrap = persist.tile([128, 128], BF16)
            scrapT = persist.tile([128, 128], BF16)
            nc.vector.memset(scrap[:], 0.0)
            nc.sync.dma_start_transpose(scrapT[:], scrap[:])
            v_sb = persist.tile([128, NKT, D], BF16)   # V[token, dim], token-chunked
            kT = persist.tile([128, NDT, S], BF16)     # K^T[dim, token]
            qT = persist.tile([128, NDT, SQ], BF16)    # Q^T[dim, token]

            # ---- d2 prebuild ----
            # a small early pool so all four q-tiles' squared-distance rows
            # build on the (otherwise idle) DVE during the projection phase
            d2p_ctx = tc.tile_pool(name="d2p", bufs=1)
            d2p = d2p_ctx.__enter__()
            mb_ctx = tc.tile_pool(name="maskbuild", bufs=1)
            mb = mb_ctx.__enter__()

            def emit_d2(qt):
                dx = mb.tile([128, S], F32, tag="dx")
                dy = mb.tile([128, S], F32, tag="dy")
                nc.vector.tensor_scalar(out=dx[:], in0=xk_b[:], scalar1=xq_col[:, qt:qt + 1],
                                        scalar2=None, op0=ALU.subtract)
                nc.vector.tensor_scalar(out=dy[:], in0=yk_b[:], scalar1=yq_col[:, qt:qt + 1],
                                        scalar2=None, op0=ALU.subtract)
                nc.scalar.square(dx[:], dx[:])
                nc.scalar.square(dy[:], dy[:])
                d2 = d2p.tile([128, S], D2_DT, tag="d2", bufs=NQT)
                nc.vector.tensor_add(d2[:], dx[:], dy[:])
                return d2

            d2_pre = {qt: emit_d2(qt) for qt in range(NQT)}

            # All projection inputs are bf16 and load up front with no
            # SBUF-reuse wait. K/Q inputs live in a pool that closes before
            # the big attention pool opens (its range is reused); V inputs
            # stay live into q-tile 0, whose head loop interleaves the 16
            # deferred V projection chains.
            vp_ctx = tc.tile_pool(name="v_in", bufs=1)
            vp = vp_ctx.__enter__()
            vps_ctx = tc.tile_pool(name="vps", bufs=1, space="PSUM")
            vps = vps_ctx.__enter__()
            wvT_sb = vp.tile([128, NDT, D], BF16)
            xvT_sb = vp.tile([128, NDT, S], BF16)
            kq_ctx = tc.tile_pool(name="kq_in", bufs=1)
            kq = kq_ctx.__enter__()
            wkT_sb = kq.tile([128, NDT, D], BF16)
            xkT_sb = kq.tile([128, NDT, S], BF16)
            wqT_sb = kq.tile([128, NDT, D], BF16)
            xqT_sb = kq.tile([128, NDT, SQ], BF16)
            _load_chunked(nc, wkT_sb, wkT, D, D)
            _load_chunked(nc, xkT_sb, xkT, D, S)
            _load_chunked(nc, wqT_sb, wqT, D, D)
            _load_chunked(nc, xqT_sb, xqT, D, SQ)
            _load_chunked(nc, wvT_sb, wvT, D, D)
            _load_chunked(nc, xvT_sb, xvT, D, S)

            with tc.tile_pool(name="kps", bufs=4, space="PSUM") as kps:
                for m in range(NDT):
                    for hf in range(2):
                        ps = kps.tile([128, 512], F32, tag="pj")
                        for t in range(NDT):
                            nc.tensor.matmul(ps[:], wkT_sb[:, t, 128 * m:128 * m + 128],
                                             xkT_sb[:, t, 512 * hf:512 * hf + 512],
                                             start=(t == 0), stop=(t == NDT - 1))
                        nc.scalar.activation(kT[:, m, 512 * hf:512 * hf + 512], ps[:],
                                             ACT.Identity, bias=bk_col[:, m:m + 1])
                for m in range(NDT):
                    ps = kps.tile([128, 512], F32, tag="pj")
                    for t in range(NDT):
                        nc.tensor.matmul(ps[:], wqT_sb[:, t, 128 * m:128 * m + 128],
                                         xqT_sb[:, t, :],
                                         start=(t == 0), stop=(t == NDT - 1))
                    nc.scalar.activation(qT[:, m, :], ps[:], ACT.Identity,
                                         bias=bq_col[:, m:m + 1])

            kq_ctx.__exit__(None, None, None)
            # big attention pool opens after kq_in closed: reuses its range
            att_ctx = tc.tile_pool(name="att", bufs=2)
            att = att_ctx.__enter__()

            def emit_vchain(k):
                hf, m = k // NKT, k % NKT
                ps = vps.tile([128, 384], F32, tag="pj")
                for t in range(NDT):
                    nc.tensor.matmul(ps[:], xvT_sb[:, t, 128 * m:128 * m + 128],
                                     wvT_sb[:, t, 384 * hf:384 * hf + 384],
                                     start=(t == 0), stop=(t == NDT - 1))
                nc.scalar.copy(v_sb[:, m, 384 * hf:384 * hf + 384], ps[:])

            if stage == "proj":
                for k in range(2 * NKT):
                    emit_vchain(k)
                with tc.tile_pool(name="dump", bufs=1) as dp:
                    t = dp.tile([128, 512], F32)
                    nc.scalar.copy(t[:], qT[:, 0, :].bitcast(F32))
                    nc.sync.dma_start(out=out[0:128, 0:512], in_=t[:])
                    t2 = dp.tile([128, 512], F32)
                    nc.scalar.copy(t2[:], kT[:, 0, 0:512].bitcast(F32))
                    nc.sync.dma_start(out=out[128:256, 0:512], in_=t2[:])
                    t3 = dp.tile([128, 512], F32)
                    nc.vector.tensor_copy(t3[:], v_sb[:, 0, 0:512])
                    nc.sync.dma_start(out=out[256:384, 0:512], in_=t3[:])

            # ---- attention ----
            if stage != "proj":
              with tc.tile_pool(name="acc_dve", bufs=2) as acc_dve, \
                 tc.tile_pool(name="acc_pool", bufs=2) as acc_pool, \
                 tc.tile_pool(name="qe_ps", bufs=1, space="PSUM") as qe_ps, \
                 tc.tile_pool(name="qk_ps", bufs=2, space="PSUM") as qk_ps, \
                 tc.tile_pool(name="av_ps", bufs=1, space="PSUM") as av_ps, \
                 tc.tile_pool(name="pt_ps", bufs=1, space="PSUM") as pt_ps:
                # --- qe -> dqe for ALL q-tiles up front (right after the Q
                # projection) so no chain ever waits on an in-order-queued dqe
                # at a q-tile boundary. Block-diagonal emb: 2 heads per
                # matmul; 64-partition sliver matmuls proved flaky on HW. ---
                dqe_pre = {}
                for qt in range(NQT):
                    qe_psum = qe_ps.tile([128, H * NUM_EMB], F32, tag="qe")
                    for m in range(NDT):
                        nc.tensor.matmul(qe_psum[:, 20 * m:20 * m + 20],
                                         qT[:, m, 128 * qt:128 * qt + 128],
                                         embT_blk[:],
                                         start=True, stop=True)
                    qe_sb = d2p.tile([128, H, NUM_EMB], F32, tag="qe_sb", bufs=2)
                    nc.scalar.copy(qe_sb[:], qe_psum[:].rearrange("p (h e) -> p h e", e=NUM_EMB))
                    dqe = d2p.tile([128, H, NUM_EMB - 1], F32, tag="dqe", bufs=NQT)
                    nc.vector.tensor_tensor(out=dqe[:], in0=qe_sb[:, :, 1:],
                                            in1=qe_sb[:, :, :-1], op=ALU.subtract)
                    dqe_pre[qt] = dqe

                for qt in range(NQT):
                    d2 = d2_pre[qt]
                    dqe = dqe_pre[qt]
                    o_tile = att.tile([128, D], F32, tag="o_tile")
                    # Software-pipelined head loop. Per head the bias term is
                    # 9 masked-scaled rows t_e = (d2 >= T_e)*dqe[h,e], each a
                    # single fast-mode tensor_scalar on DVE. Accumulation of
                    # the t_e onto the qk logits happens per-head on one of:
                    #   - PE: identity-stationary matmuls accumulate the t_e
                    #     directly into the qk PSUM (most heads),
                    #   - DVE/Pool: tensor-tensor add chains in SBUF.
                    # PE-consuming stages are emitted LAG slots late so the
                    # in-order PE stream never waits on the current head.
                    lag = 4 if qt == 0 else 2
                    lag_f = lag + 1   # recip/final run one slot after AV
                    # binary step masks for this q-tile, shared by all heads
                    masks = att.tile([128, NUM_EMB - 1, S], BF16, tag="masks", bufs=2)
                    for e in range(NUM_EMB - 1):
                        nc.vector.tensor_scalar(out=masks[:, e, :], in0=d2[:],
                                                scalar1=THRESH2[e], scalar2=None,
                                                op0=ALU.is_ge)
                    tes = {}
                    state = {}
                    avs = {}
                    for slot in range(H + lag_f):
                        if slot < H:
                            h = slot
                            off = (64 * h) % 128
                            pe_acc = h not in DVE_CHAIN_HEADS and h not in POOL_CHAIN_HEADS
                            if pe_acc:
                                # per-head scaled rows t_e = dqe[h,e]*mask_e
                                # (4x-mode bf16 tensor_scalar on DVE)
                                te = att.tile([128, NUM_EMB - 1, S], BF16,
                                              tag="te", bufs=2)
                                for e in range(NUM_EMB - 1):
                                    nc.vector.tensor_scalar(
                                        out=te[:, e, :], in0=masks[:, e, :],
                                        scalar1=dqe[:, h, e:e + 1], scalar2=None,
                                        op0=ALU.mult)
                            else:
                                te = None
                            tes[h] = te
                            # --- logits = q.k^T (PE-acc heads leave the
                            # accumulation group open for the mask adds) ---
                            qk = qk_ps.tile([128, S], F32, tag="qk")
                            for hf in range(2):
                                nc.tensor.matmul(qk[:, 512 * hf:512 * hf + 512],
                                                 qT[off:off + 64, h // 2, 128 * qt:128 * qt + 128],
                                                 kT[off:off + 64, h // 2, 512 * hf:512 * hf + 512],
                                                 start=True, stop=not pe_acc)
                            if qt == 0 and slot < NKT:
                                emit_vchain(2 * slot)
                                emit_vchain(2 * slot + 1)
                            if not pe_acc:
                                # --- bias via 9 chained masked MACs on DVE
                                # (the original scalar_tensor_tensor form) ---
                                src = qk
                                for e in range(NUM_EMB - 1):
                                    acc = acc_dve.tile([128, S], F32, tag="acc")
                                    nc.vector.scalar_tensor_tensor(
                                        out=acc[:], in0=masks[:, e, :],
                                        scalar=dqe[:, h, e:e + 1],
                                        in1=src[:], op0=ALU.mult, op1=ALU.add)
                                    src = acc
                                state[h] = (src, None)
                            state.setdefault(h, (None, qk))
                        if 1 <= slot < H + 1:
                            # one slot after qk: PE accumulates dqe_e * mask_e
                            # onto qk via diag stationaries, then exp +
                            # row-sum + transpose
                            h1 = slot - 1
                            te = tes.pop(h1)
                            src, qk = state.pop(h1)
                            if src is None:
                                for e in range(NUM_EMB - 1):
                                    for hf in range(2):
                                        nc.tensor.matmul(
                                            qk[:, 512 * hf:512 * hf + 512],
                                            ident[:],
                                            te[:, e, 512 * hf:512 * hf + 512],
                                            start=False, stop=(e == NUM_EMB - 2))
                                src = qk
                            p_sb = att.tile([128, S], BF16, tag="p", bufs=3)
                            den = att.tile([128, 1], F32, tag="den", bufs=6)
                            nc.scalar.activation(p_sb[:], src[:], ACT.Exp, scale=0.125,
                                                 accum_out=den[:])
                            # P transpose on the PE (the xbar dma-transpose
                            # path intermittently corrupts pT on HW)
                            ptp = pt_ps.tile([128, NKT, 128], BF16, tag="ptp")
                            for c in range(NKT):
                                nc.tensor.transpose(ptp[:, c, :], p_sb[:, 128 * c:128 * c + 128], ident[:])
                            pT = att.tile([128, NKT, 128], BF16, tag="pT", bufs=5)
                            nc.scalar.copy(pT[:], ptp[:])
                            state[h1] = (pT, den)
                        if lag <= slot < H + lag:
                            h2 = slot - lag
                            pT, den = state.pop(h2)
                            # --- av = P^T . V_h ---
                            av = av_ps.tile([128, DK], F32, tag="av")
                            for c in range(NKT):
                                nc.tensor.matmul(av[:], pT[:, c, :],
                                                 v_sb[:, c, 64 * h2:64 * h2 + 64],
                                                 start=(c == 0), stop=(c == NKT - 1))
                            avs[h2] = (av, den)
                        if slot >= lag_f:
                            h3 = slot - lag_f
                            av, den = avs.pop(h3)
                            # --- out_h = av / den + bv_h ---
                            recip = att.tile([128, 1], F32, tag="recip")
                            nc.vector.reciprocal(recip[:], den[:])
                            nc.vector.scalar_tensor_tensor(
                                out=o_tile[:, 64 * h3:64 * h3 + 64], in0=av[:], scalar=recip[:],
                                in1=bv_b[:, 64 * h3:64 * h3 + 64], op0=ALU.mult, op1=ALU.add)
                    nc.sync.dma_start(out=out[128 * qt:128 * qt + 128, :], in_=o_tile[:])
            att_ctx.__exit__(None, None, None)
            vps_ctx.__exit__(None, None, None)
            vp_ctx.__exit__(None, None, None)
            mb_ctx.__exit__(None, None, None)
            d2p_ctx.__exit__(None, None, None)
    nc.compile()
    return nc


_NC_CACHE = {}


def _get_nc():
    if "nc" not in _NC_CACHE:
        _NC_CACHE["nc"] = build_nc()
    return _NC_CACHE["nc"]


def _make_in_maps(inputs):
    q = np.ascontiguousarray(np.asarray(inputs["query"], dtype=np.float32))
    k = np.ascontiguousarray(np.asarray(inputs["key"], dtype=np.float32))
    v = np.ascontiguousarray(np.asarray(inputs["value"], dtype=np.float32))
    tp = np.ascontiguousarray(np.asarray(inputs["tile_positions"], dtype=np.float32))
    import ml_dtypes
    WqT = np.ascontiguousarray(
        np.asarray(inputs["Wq"], dtype=np.float32).T.astype(ml_dtypes.bfloat16))
    WkT = np.ascontiguousarray(
        np.asarray(inputs["Wk"], dtype=np.float32).T.astype(ml_dtypes.bfloat16))
    WvT = np.ascontiguousarray(
        np.asarray(inputs["Wv"], dtype=np.float32).T.astype(ml_dtypes.bfloat16))
    bq = np.ascontiguousarray(np.asarray(inputs["bq"], dtype=np.float32))
    bk = np.ascontiguousarray(np.asarray(inputs["bk"], dtype=np.float32))
    bv = np.ascontiguousarray(np.asarray(inputs["bv"], dtype=np.float32))
    embT = np.ascontiguousarray(np.asarray(inputs["emb_k"], dtype=np.float32).T.astype(ml_dtypes.bfloat16))

    xqT_b = [np.ascontiguousarray(q[b].T.astype(ml_dtypes.bfloat16)) for b in range(B)]
    xkT_b = [np.ascontiguousarray(k[b].T.astype(ml_dtypes.bfloat16)) for b in range(B)]
    xvT_b = [np.ascontiguousarray(v[b].T.astype(ml_dtypes.bfloat16)) for b in range(B)]

    in_maps = []
    for c in range(NCORES):
        b, qh = c // 2, c % 2
        in_maps.append({
            "xqT": np.ascontiguousarray(xqT_b[b][:, qh * SQ:(qh + 1) * SQ]),
            "xkT": xkT_b[b], "xvT": xvT_b[b],
            "pos": tp[b],
            "posq": np.ascontiguousarray(tp[b, qh * SQ:(qh + 1) * SQ]),
            "wqT": WqT, "wkT": WkT, "wvT": WvT,
            "bq": bq, "bk": bk, "bv": bv,
            "embT_h": embT,
        })
    return in_maps


def kernel(query, key, value, tile_positions, Wq, bq, Wk, bk, Wv, bv, emb_k):
    inputs = {"query": query, "key": key, "value": value,
              "tile_positions": tile_positions,
              "Wq": Wq, "bq": bq, "Wk": Wk, "bk": bk, "Wv": Wv, "bv": bv,
              "emb_k": emb_k}
    nc = _get_nc()
    in_maps = _make_in_maps(inputs)
    res = run_bass_kernel_spmd(nc, in_maps, core_ids=list(range(NCORES)))
    out = np.empty((B, S, D), np.float32)
    for c in range(NCORES):
        b, qh = c // 2, c % 2
        out[b, qh * SQ:(qh + 1) * SQ] = res.results[c]["out"]
    return out
